# revision 1
# baseline (speedup 1.0000x reference)
"""GCN link-predictor kernel for 8 Trainium2 NeuronCores (Bass/Tile).

Strategy (SPMD, single program on 8 cores, no core-dependent addressing):
  - Host: append self loops, sort edges by dst, partition nodes into 8
    contiguous ranges (12500/core, padded to 12544 = 98 tiles of 128).
    Core q owns all edges whose dst lies in its range, grouped per
    128-node destination tile, padded to a uniform K chunks of 128 edges.
  - deg:   per-chunk one-hot matrices W[e, dst_local] = ew[e] (built on DVE
           from an iota constant via fused is_equal*mult tensor_scalar),
           deg_tile = sum_k W_k^T @ 1.  dinv = 1/sqrt(max(deg,1)).
           AllGather dinv shards -> full dinv table in SBUF.
  - layer: h' = dinv * (x @ W)  (full table per core; lhsT = host-side
           pre-transposed x tiles so the GEMM emits node-major tiles),
           stored bf16 in DRAM.  Aggregation per owned dst tile:
           indirect-DMA gather of 128 h' rows per chunk + one-hot matmul
           accumulated in PSUM; evict relu(dinv*psum + b).
           Layer-1 output is PE-transposed and AllGathered as [1024,12544]
           so layer-2 GEMM can slice lhsT tiles directly; layer-2 output is
           AllGathered node-major for the label gathers.
  - labels: gather out2[el0], out2[el1], res = sum(a*b*w_vec) + sum(lin_b)
           where w_vec = lin_W @ 1 (the final linear collapses to a
           weighted inner product).
"""

import os
import sys

import numpy as np

for _p in ("/opt/trn_rl_repo",):
    if _p not in sys.path:
        sys.path.insert(0, _p)

import ml_dtypes  # noqa: E402

import concourse.bacc as bacc  # noqa: E402
import concourse.bass as bass  # noqa: E402
import concourse.mybir as mybir  # noqa: E402
from concourse.bass import IndirectOffsetOnAxis  # noqa: E402
from concourse.bass_utils import run_bass_kernel_spmd  # noqa: E402
from concourse.tile import TileContext  # noqa: E402

P = 128
NC = 8
BF = mybir.dt.bfloat16
F32 = mybir.dt.float32
I32 = mybir.dt.int32

LAST_EXEC_NS = None
LAST_RESULTS = None


class Cfg:
    def __init__(self, n_nodes, n_labels):
        assert n_nodes % NC == 0
        self.n_nodes = n_nodes
        self.nodes_per_core = n_nodes // NC
        self.tiles_per_core = -(-self.nodes_per_core // P)
        self.n_loc = self.tiles_per_core * P
        self.n_pad = NC * self.n_loc
        self.n_labels = n_labels
        self.lab_per_core = -(-n_labels // NC)
        self.lab_chunks = -(-self.lab_per_core // P)


FULL = Cfg(100000, 200000)


# ---------------------------------------------------------------- host prep


def _pad_ids(cfg, ids):
    q, l = np.divmod(ids, cfg.nodes_per_core)
    q = np.minimum(q, NC - 1)
    l = ids - q * cfg.nodes_per_core
    return q * cfg.n_loc + l, q, l


def preprocess(cfg, x, edge_index, edge_weight, edge_label_index):
    n = cfg.n_nodes
    src = np.concatenate([edge_index[0], np.arange(n)]).astype(np.int64)
    dst = np.concatenate([edge_index[1], np.arange(n)]).astype(np.int64)
    ew = np.concatenate(
        [edge_weight.astype(np.float32), np.ones(n, np.float32)]
    )

    src_pad, _, _ = _pad_ids(cfg, src)
    _, dq, dl = _pad_ids(cfg, dst)
    T = cfg.tiles_per_core
    bucket = dq * T + dl // P
    counts = np.bincount(bucket, minlength=NC * T).reshape(NC, T)
    # per-tile chunk count: max over cores (keeps the SPMD program uniform
    # while minimizing total chunks; walrus caps indirect DMAs at ~4096)
    K_arr = np.maximum(1, -(-counts.max(axis=0) // P))  # [T]
    off = np.zeros(T + 1, np.int64)
    off[1:] = np.cumsum(K_arr)
    C = int(off[-1])

    order = np.argsort(bucket, kind="stable")
    sb = bucket[order]
    starts = np.zeros(NC * T + 1, np.int64)
    starts[1:] = np.cumsum(counts.reshape(-1))
    pos = np.arange(len(order)) - starts[sb]
    dest_core = sb // T
    dest_flat = off[sb % T] * P + pos  # within-core flat edge slot

    srci = np.zeros((NC, C * P), np.int32)
    dstl = np.zeros((NC, C * P), np.float32)
    ewp = np.zeros((NC, C * P), np.float32)
    srci[dest_core, dest_flat] = src_pad[order]
    dstl[dest_core, dest_flat] = (dl % P)[order]
    ewp[dest_core, dest_flat] = ew[order]

    # [core, C*P] -> [core, 128, C]   (partition = edge slot within chunk)
    def to_pc(a, dt):
        return np.ascontiguousarray(
            a.reshape(NC, C, P).transpose(0, 2, 1)
        ).astype(dt)

    n_gather = 2 * C + 2 * cfg.lab_chunks
    assert n_gather <= 4080, f"indirect DMA budget exceeded: {n_gather}"


    srci = to_pc(srci, np.int32)
    meta = np.concatenate(
        [to_pc(dstl, np.float32), to_pc(ewp, np.float32)], axis=-1
    ).astype(ml_dtypes.bfloat16)

    # labels
    el_pad, _, _ = _pad_ids(cfg, edge_label_index.astype(np.int64))
    LC = cfg.lab_chunks
    el0 = np.zeros((NC, LC * P), np.int32)
    el1 = np.zeros((NC, LC * P), np.int32)
    lpc = cfg.lab_per_core
    for q in range(NC):
        lo, hi = q * lpc, min((q + 1) * lpc, cfg.n_labels)
        el0[q, : hi - lo] = el_pad[0, lo:hi]
        el1[q, : hi - lo] = el_pad[1, lo:hi]
    el0 = np.ascontiguousarray(el0.reshape(NC, LC, P).transpose(0, 2, 1))
    el1 = np.ascontiguousarray(el1.reshape(NC, LC, P).transpose(0, 2, 1))

    # node features, padded + transposed
    pid_all, _, _ = _pad_ids(cfg, np.arange(n))
    x_pad = np.zeros((cfg.n_pad, P), np.float32)
    x_pad[pid_all] = x
    xT = np.ascontiguousarray(x_pad.T).astype(ml_dtypes.bfloat16)

    kmax = int(K_arr.max())
    iota_rep = np.tile(
        np.arange(P, dtype=np.float32)[None, :], (P, kmax)
    ).astype(ml_dtypes.bfloat16)
    return dict(srci=srci, meta=meta, el0=el0, el1=el1, xT=xT,
                K_arr=[int(v) for v in K_arr], iota_rep=iota_rep)


# ------------------------------------------------------------- bass program


def build_program(cfg, K_arr, linb_sum, phase=99):
    K_off = [0]
    for v in K_arr:
        K_off.append(K_off[-1] + v)
    KMAX = max(K_arr)
    T = cfg.tiles_per_core
    C = K_off[-1]
    NPAD, NLOC, LC = cfg.n_pad, cfg.n_loc, cfg.lab_chunks
    GT = NC * T  # global tiles
    rg = [list(range(NC))]

    nc = bacc.Bacc(None, target_bir_lowering=False, debug=False)

    xT = nc.declare_dram_parameter("xT", [P, NPAD], BF, False)
    srci_d = nc.declare_dram_parameter("srci", [P, C], I32, False)
    meta_d = nc.declare_dram_parameter("meta", [P, 2 * C], BF, False)
    el0_d = nc.declare_dram_parameter("el0", [P, LC], I32, False)
    el1_d = nc.declare_dram_parameter("el1", [P, LC], I32, False)
    iota_d = nc.declare_dram_parameter("iota", [P, KMAX * P], BF, False)
    ident_d = nc.declare_dram_parameter("ident", [P, P], BF, False)
    w1_d = nc.declare_dram_parameter("w1", [P, P], BF, False)
    w2_d = nc.declare_dram_parameter("w2", [P, P], BF, False)
    b1_d = nc.declare_dram_parameter("b1bc", [P, P], F32, False)
    b2_d = nc.declare_dram_parameter("b2bc", [P, P], F32, False)
    wv_d = nc.declare_dram_parameter("wvbc", [P, P], F32, False)
    res_d = nc.declare_dram_parameter("res", [P, LC], F32, True)

    htab = nc.dram_tensor("htab", [NPAD, P], BF)
    dinv_sh = nc.dram_tensor("dinv_sh", [1, NLOC], F32)
    dinv_ag = nc.dram_tensor("dinv_ag", [NC, NLOC], F32, addr_space="Shared")
    o1t_sh = nc.dram_tensor("o1t_sh", [P, NLOC], BF)
    o1t_ag = nc.dram_tensor("o1t_ag", [NC * P, NLOC], BF, addr_space="Shared")
    o2_sh = nc.dram_tensor("o2_sh", [NLOC, P], BF)
    o2_ag = nc.dram_tensor("o2_ag", [NPAD, P], BF)

    AF = mybir.ActivationFunctionType
    OP = mybir.AluOpType

    with TileContext(nc) as tc:
        with (
            tc.tile_pool(name="const", bufs=1) as cp,
            tc.tile_pool(name="wtile", bufs=6) as wp,
            tc.tile_pool(name="htile", bufs=8) as hp,
            tc.tile_pool(name="gemm", bufs=6) as gp,
            tc.tile_pool(name="evict", bufs=4) as ep,
            tc.tile_pool(name="lab", bufs=8) as lp,
            tc.tile_pool(name="ps_deg", bufs=2, space="PSUM") as psd,
            tc.tile_pool(name="ps_gemm", bufs=2, space="PSUM") as psg,
            tc.tile_pool(name="ps_agg", bufs=2, space="PSUM") as psa,
            tc.tile_pool(name="ps_tr", bufs=2, space="PSUM") as pst,
        ):
            # ---- persistent SBUF ----
            srci_sb = cp.tile([P, C], I32)
            nc.sync.dma_start(out=srci_sb[:], in_=srci_d[:, :])
            meta_sb = cp.tile([P, 2 * C], BF)
            nc.sync.dma_start(out=meta_sb[:], in_=meta_d[:, :])
            el0_sb = cp.tile([P, LC], I32)
            nc.sync.dma_start(out=el0_sb[:], in_=el0_d[:, :])
            el1_sb = cp.tile([P, LC], I32)
            nc.sync.dma_start(out=el1_sb[:], in_=el1_d[:, :])
            iota_sb = cp.tile([P, KMAX * P], BF)
            nc.sync.dma_start(out=iota_sb[:], in_=iota_d[:, :])
            ident_sb = cp.tile([P, P], BF)
            nc.sync.dma_start(out=ident_sb[:], in_=ident_d[:, :])
            w1_sb = cp.tile([P, P], BF)
            nc.sync.dma_start(out=w1_sb[:], in_=w1_d[:, :])
            w2_sb = cp.tile([P, P], BF)
            nc.sync.dma_start(out=w2_sb[:], in_=w2_d[:, :])
            b1_sb = cp.tile([P, P], F32)
            nc.sync.dma_start(out=b1_sb[:], in_=b1_d[:, :])
            b2_sb = cp.tile([P, P], F32)
            nc.sync.dma_start(out=b2_sb[:], in_=b2_d[:, :])
            wv_sb = cp.tile([P, P], F32)
            nc.sync.dma_start(out=wv_sb[:], in_=wv_d[:, :])
            ones_sb = cp.tile([P, 1], BF)
            nc.vector.memset(ones_sb[:], 1.0)
            deg_sb = cp.tile([P, T], F32)
            dinv_own = cp.tile([P, T], F32)
            dinvF = cp.tile([P, GT], F32)
            res_sb = cp.tile([P, LC], F32)

            iota3 = iota_sb[:].rearrange("p (g e) -> p g e", e=P)

            def build_w(lt):
                # one-hot W for all chunks of tile lt in two batched ops
                K = K_arr[lt]
                c0 = K_off[lt]
                w = wp.tile([P, KMAX * P], BF, tag="w")
                w3 = w[:, : K * P].rearrange("p (g e) -> p g e", e=P)
                nc.vector.tensor_tensor(
                    out=w3,
                    in0=iota3[:, :K, :],
                    in1=meta_sb[:, c0 : c0 + K].to_broadcast([P, K, P]),
                    op=OP.is_equal,
                )
                nc.vector.tensor_tensor(
                    out=w3,
                    in0=w3,
                    in1=meta_sb[:, C + c0 : C + c0 + K].to_broadcast(
                        [P, K, P]
                    ),
                    op=OP.mult,
                )
                return w

            # ---- deg pass ----
            for lt in range(T):
                pd = psd.tile([P, 1], F32)
                w = build_w(lt)
                K = K_arr[lt]
                for k in range(K):
                    nc.tensor.matmul(
                        out=pd[:],
                        lhsT=w[:, k * P : (k + 1) * P],
                        rhs=ones_sb[:],
                        start=(k == 0),
                        stop=(k == K - 1),
                    )
                nc.scalar.activation(deg_sb[:, lt : lt + 1], pd[:], AF.Copy)
            # dinv = 1/sqrt(max(deg,1));  deg>=1 for real nodes (self loop),
            # dead padding nodes get deg=1 to avoid inf/NaN.
            nc.vector.tensor_scalar_max(deg_sb[:], deg_sb[:], 1.0)
            rec_sb = cp.tile([P, T], F32)
            nc.vector.reciprocal(rec_sb[:], deg_sb[:])
            nc.scalar.activation(dinv_own[:], rec_sb[:], AF.Sqrt)
            nc.sync.dma_start(
                out=dinv_sh.ap().rearrange("a (p l) -> (a p) l", p=P),
                in_=dinv_own[:],
            )
            nc.gpsimd.collective_compute(
                "AllGather",
                OP.bypass,
                replica_groups=rg,
                ins=[dinv_sh[:, :]],
                outs=[dinv_ag[:, :]],
            )
            nc.sync.dma_start(
                out=dinvF[:].rearrange("p (q l) -> p q l", q=NC),
                in_=dinv_ag.ap().rearrange("q (p l) -> p q l", p=P),
            )
            if phase <= 1:
                nc.sync.dma_start(out=res_d[:, :], in_=dinvF[:, :LC])

            # ---- h' table GEMM pass ----
            def gemm_pass(layer):
                w_sb = w1_sb if layer == 1 else w2_sb
                for t in range(GT):
                    lhsT = gp.tile([P, P], BF, tag="lhsT")
                    if layer == 1:
                        nc.sync.dma_start(
                            out=lhsT[:], in_=xT[:, t * P : (t + 1) * P]
                        )
                    else:
                        q, lt = divmod(t, T)
                        nc.sync.dma_start(
                            out=lhsT[:],
                            in_=o1t_ag[
                                q * P : (q + 1) * P, lt * P : (lt + 1) * P
                            ],
                        )
                    pg = psg.tile([P, P], F32)
                    nc.tensor.matmul(
                        out=pg[:], lhsT=lhsT[:], rhs=w_sb[:],
                        start=True, stop=True,
                    )
                    hbf = gp.tile([P, P], BF, tag="hbf")
                    nc.scalar.activation(
                        hbf[:], pg[:], AF.Copy, scale=dinvF[:, t : t + 1]
                    )
                    nc.sync.dma_start(
                        out=htab[t * P : (t + 1) * P, :], in_=hbf[:]
                    )

            # ---- aggregation pass over owned dst tiles ----
            def agg_pass(layer):
                b_sb = b1_sb if layer == 1 else b2_sb
                for lt in range(T):
                    pa = psa.tile([P, P], F32)
                    w = build_w(lt)
                    K = K_arr[lt]
                    for k in range(K):
                        c = K_off[lt] + k
                        h = hp.tile([P, P], BF, tag="h")
                        nc.gpsimd.indirect_dma_start(
                            out=h[:],
                            out_offset=None,
                            in_=htab[:, :],
                            in_offset=IndirectOffsetOnAxis(
                                ap=srci_sb[:, c : c + 1], axis=0
                            ),
                        )
                        nc.tensor.matmul(
                            out=pa[:],
                            lhsT=w[:, k * P : (k + 1) * P],
                            rhs=h[:],
                            start=(k == 0),
                            stop=(k == K - 1),
                        )
                    t1 = ep.tile([P, P], F32, tag="t1")
                    nc.scalar.activation(
                        t1[:], pa[:], AF.Copy,
                        scale=dinv_own[:, lt : lt + 1],
                    )
                    nc.vector.tensor_tensor(
                        out=t1[:], in0=t1[:], in1=b_sb[:], op=OP.add
                    )
                    obf = ep.tile([P, P], BF, tag="obf")
                    nc.scalar.activation(obf[:], t1[:], AF.Relu)
                    if layer == 1:
                        pt = pst.tile([P, P], BF)
                        nc.tensor.transpose(
                            out=pt[:], in_=obf[:], identity=ident_sb[:]
                        )
                        otb = ep.tile([P, P], BF, tag="otb")
                        nc.scalar.activation(otb[:], pt[:], AF.Copy)
                        nc.sync.dma_start(
                            out=o1t_sh[:, lt * P : (lt + 1) * P], in_=otb[:]
                        )
                    else:
                        nc.sync.dma_start(
                            out=o2_sh[lt * P : (lt + 1) * P, :], in_=obf[:]
                        )

            if phase >= 2:
                gemm_pass(1)
            if phase == 2:
                hprobe = cp.tile([P, P], BF)
                nc.sync.dma_start(out=hprobe[:], in_=htab[0:P, :])
                probe_f = cp.tile([P, P], F32)
                nc.vector.tensor_copy(probe_f[:], hprobe[:])
                nc.sync.dma_start(out=res_d[:, :], in_=probe_f[:, :LC])
            if phase >= 3:
                agg_pass(1)
            if phase == 3:
                oprobe = cp.tile([P, P], BF)
                nc.sync.dma_start(out=oprobe[:], in_=o1t_sh[:, 0:P])
                oprobe_f = cp.tile([P, P], F32)
                nc.vector.tensor_copy(oprobe_f[:], oprobe[:])
                nc.sync.dma_start(out=res_d[:, :], in_=oprobe_f[:, :LC])
            if phase >= 4:
                nc.gpsimd.collective_compute(
                    "AllGather",
                    OP.bypass,
                    replica_groups=rg,
                    ins=[o1t_sh[:, :]],
                    outs=[o1t_ag[:, :]],
                )
            if phase == 4:
                oprobe = cp.tile([P, P], BF)
                nc.sync.dma_start(out=oprobe[:], in_=o1t_ag[0:P, 0:P])
                oprobe_f = cp.tile([P, P], F32)
                nc.vector.tensor_copy(oprobe_f[:], oprobe[:])
                nc.sync.dma_start(out=res_d[:, :], in_=oprobe_f[:, :LC])
            if phase >= 5:
                gemm_pass(2)
            if phase >= 5:
                agg_pass(2)
                nc.gpsimd.collective_compute(
                    "AllGather",
                    OP.bypass,
                    replica_groups=rg,
                    ins=[o2_sh[:, :]],
                    outs=[o2_ag[:, :]],
                )

            # ---- label pass ----
            for c in range(LC if phase >= 6 else 0):
                a = lp.tile([P, P], BF, tag="a")
                nc.gpsimd.indirect_dma_start(
                    out=a[:],
                    out_offset=None,
                    in_=o2_ag[:, :],
                    in_offset=IndirectOffsetOnAxis(
                        ap=el0_sb[:, c : c + 1], axis=0
                    ),
                )
                b = lp.tile([P, P], BF, tag="b")
                nc.gpsimd.indirect_dma_start(
                    out=b[:],
                    out_offset=None,
                    in_=o2_ag[:, :],
                    in_offset=IndirectOffsetOnAxis(
                        ap=el1_sb[:, c : c + 1], axis=0
                    ),
                )
                prod = lp.tile([P, P], F32, tag="prod")
                nc.vector.tensor_tensor(
                    out=prod[:], in0=a[:], in1=b[:], op=OP.mult
                )
                scr = lp.tile([P, P], F32, tag="scr")
                nc.vector.tensor_tensor(
                    out=scr[:], in0=prod[:], in1=wv_sb[:], op=OP.mult
                )
                nc.vector.reduce_sum(
                    res_sb[:, c : c + 1], scr[:], axis=mybir.AxisListType.X
                )
            if phase >= 6:
                nc.vector.tensor_scalar_add(
                    res_sb[:], res_sb[:], float(linb_sum)
                )
                nc.sync.dma_start(out=res_d[:, :], in_=res_sb[:])

    nc.finalize()
    return nc


# ------------------------------------------------------------------ driver


def make_in_maps(cfg, prep, W1, b1, W2, b2, lin_W, lin_b):
    consts = dict(
        xT=prep["xT"],
        iota=prep["iota_rep"],
        ident=np.eye(P, dtype=np.float32).astype(ml_dtypes.bfloat16),
        w1=W1.astype(np.float32).astype(ml_dtypes.bfloat16),
        w2=W2.astype(np.float32).astype(ml_dtypes.bfloat16),
        b1bc=np.tile(b1.astype(np.float32)[None, :], (P, 1)),
        b2bc=np.tile(b2.astype(np.float32)[None, :], (P, 1)),
        wvbc=np.tile(
            lin_W.astype(np.float32).sum(axis=1)[None, :], (P, 1)
        ),
    )
    in_maps = []
    for q in range(NC):
        m = dict(consts)
        m.update(
            srci=prep["srci"][q],
            meta=prep["meta"][q],
            el0=prep["el0"][q],
            el1=prep["el1"][q],
        )
        in_maps.append(m)
    return in_maps


def assemble_output(cfg, results):
    outs = []
    for q in range(NC):
        r = np.asarray(results[q]["res"], np.float32)  # [128, LC]
        outs.append(r.T.reshape(-1)[: cfg.lab_per_core])
    return np.concatenate(outs)[: cfg.n_labels].astype(np.float32)


def run(cfg, x, edge_index, edge_weight, edge_label_index,
        W1, b1, W2, b2, lin_W, lin_b, trace=False, phase=99):
    global LAST_EXEC_NS, LAST_RESULTS
    prep = preprocess(cfg, np.asarray(x), np.asarray(edge_index),
                      np.asarray(edge_weight), np.asarray(edge_label_index))
    linb_sum = float(np.asarray(lin_b, np.float64).sum())
    nc = build_program(cfg, prep["K_arr"], linb_sum, phase=phase)
    in_maps = make_in_maps(cfg, prep, W1, b1, W2, b2, lin_W, lin_b)
    res = run_bass_kernel_spmd(
        nc, in_maps, list(range(NC)), trace=trace
    )
    LAST_EXEC_NS = res.exec_time_ns
    LAST_RESULTS = res
    return assemble_output(cfg, res.results)


def kernel(x, edge_index, edge_weight, edge_label_index,
           W1, b1, W2, b2, lin_W, lin_b):
    trace = bool(os.environ.get("KERNEL_TRACE"))
    return run(FULL, x, edge_index, edge_weight, edge_label_index,
               W1, b1, W2, b2, lin_W, lin_b, trace=trace)



# revision 20
# speedup vs baseline: 1.3007x; 1.3007x over previous
"""GCN link-predictor kernel for 8 Trainium2 NeuronCores (Bass/Tile).

Strategy (SPMD, dst-sharded, v2):
  - Host: append self loops, assign each edge to the core owning its dst,
    group per 128-node dst tile, sort each tile's edges by src range
    (32768 rows = int16 dma_gather window), pad each (tile, range) to
    whole 128-edge chunks.  Ship the one-hot scatter matrices W_ew
    (W[e, dstl] = ew) pre-built in bf16, plus int16 local gather indices
    (16-partition wrapped) -- so the device does no one-hot construction.
  - Device per layer:  gather table rows with dma_gather (one call per
    supertile x src-range; ~1us SWDGE overhead amortized over thousands
    of rows), then chunk matmuls vs W_ew accumulate dst-tile partials in
    PSUM.  Layer1 table = dinv (.) (x @ W1) built from the core's own
    shard (lhsT = host-pre-transposed xT) and AllGathered; layer2 table
    = dinv (.) out1 produced directly at the layer1 evict (aggregate-
    then-GEMM reordering: (A X) W = A (X W)), so no full-table GEMM pass
    and no global dinv exchange -- all dinv folds are own-shard.
  - Labels: pairs sorted by (range(el0), range(el1)) into 16 groups so
    both sides gather via dma_gather; score = sum(a*b*wv) + sum(lin_b)
    with wv = lin_W @ 1.  Host un-permutes the result.
"""

import os
import sys

import numpy as np

for _p in ("/opt/trn_rl_repo",):
    if _p not in sys.path:
        sys.path.insert(0, _p)

import ml_dtypes  # noqa: E402

import concourse.bacc as bacc  # noqa: E402
import concourse.bass as bass  # noqa: E402
import concourse.mybir as mybir  # noqa: E402
from concourse.bass_utils import run_bass_kernel_spmd  # noqa: E402
from concourse.tile import TileContext  # noqa: E402

P = 128
NC = 8
RS = 32768  # dma_gather int16 window (rows)
SUP = 4    # dst tiles per supertile (gather-call granularity)
GMAX = 8   # max 128-row blocks per dma_gather call (SWDGE ring limit)
BF = mybir.dt.bfloat16
F32 = mybir.dt.float32
I16 = mybir.dt.int16

LAST_EXEC_NS = None
LAST_RESULTS = None


class Cfg:
    def __init__(self, n_nodes, n_labels, rs=RS):
        assert n_nodes % NC == 0
        self.n_nodes = n_nodes
        self.nodes_per_core = n_nodes // NC
        self.tiles_per_core = -(-self.nodes_per_core // P)
        self.n_loc = self.tiles_per_core * P
        self.n_pad = NC * self.n_loc
        self.n_labels = n_labels
        self.lab_per_core = -(-n_labels // NC)
        self.rs = rs
        self.nrg = -(-self.n_pad // rs)


FULL = Cfg(100000, 200000)


# ---------------------------------------------------------------- host prep


def _pad_ids(cfg, ids):
    q = np.minimum(ids // cfg.nodes_per_core, NC - 1)
    l = ids - q * cfg.nodes_per_core
    return q * cfg.n_loc + l


def _wrap16(flat_idx):
    # [n] -> [128, n//16]: idx j at [j%16, j//16], replicated to 128 parts
    n = len(flat_idx)
    assert n % 16 == 0
    a = np.zeros((16, n // 16), np.int16)
    a[np.arange(n) % 16, np.arange(n) // 16] = flat_idx
    return np.tile(a, (8, 1))


def preprocess(cfg, x, edge_index, edge_weight, edge_label_index):
    n = cfg.n_nodes
    T, NRG = cfg.tiles_per_core, cfg.nrg
    src = np.concatenate([edge_index[0], np.arange(n)]).astype(np.int64)
    dst = np.concatenate([edge_index[1], np.arange(n)]).astype(np.int64)
    ew = np.concatenate(
        [edge_weight.astype(np.float32), np.ones(n, np.float32)]
    )

    src_pad = _pad_ids(cfg, src)
    dst_pad = _pad_ids(cfg, dst)
    dq, dl = np.divmod(dst_pad, cfg.n_loc)
    dt_ = dl // P          # dst tile within core
    dloc = dl % P          # dst row within tile
    srange = src_pad // cfg.rs

    # chunk counts per (tile, range): max over cores for SPMD uniformity
    key = (dq * T + dt_) * NRG + srange
    counts = np.bincount(key, minlength=NC * T * NRG).reshape(NC, T, NRG)
    Kr = -(-counts.max(axis=0) // P)  # [T, NRG] blocks (may be 0)

    # supertile slab layout: for each supertile, range-major block order
    sups = []
    boff = np.zeros((T, NRG), np.int64)  # global block offset of (t, r)
    gcol = 0  # running gidx column offset
    nblk = 0
    for t0 in range(0, T, SUP):
        tiles = list(range(t0, min(t0 + SUP, T)))
        calls = []
        slab0 = nblk
        tile_chunks = {t: [] for t in tiles}
        for r in range(NRG):
            nb = int(sum(Kr[t, r] for t in tiles))
            if nb == 0:
                continue
            call_start = nblk
            for t in tiles:
                boff[t, r] = nblk
                tile_chunks[t].extend(range(nblk, nblk + int(Kr[t, r])))
                nblk += int(Kr[t, r])
            calls.append(dict(r=r, nb=nb, blk0=call_start, gcol=gcol))
            gcol += nb * 8  # nb*128/16 cols
        sups.append(dict(tiles=tiles, calls=calls, slab0=slab0,
                         nblk=nblk - slab0,
                         tile_chunks={t: [b - slab0 for b in tile_chunks[t]]
                                      for t in tiles}))
    C = nblk

    # slot assignment for every edge
    order = np.argsort(key, kind="stable")
    sk = key[order]
    starts = np.zeros(NC * T * NRG + 1, np.int64)
    starts[1:] = np.cumsum(counts.reshape(-1))
    pos = np.arange(len(order)) - starts[sk]
    e_core = order * 0 + dq[order]
    e_t, e_r = dt_[order], srange[order]
    blk = boff[e_t, e_r] + pos // P
    part = pos % P

    wew_lin = blk * (P * P) + part * P + dloc[order]
    ew_bf = ew[order].astype(ml_dtypes.bfloat16)
    wew = []
    for q in range(NC):
        m = e_core == q
        arr = np.zeros(C * P * P, ml_dtypes.bfloat16)
        arr[wew_lin[m]] = ew_bf[m]
        wew.append(np.ascontiguousarray(
            arr.reshape(C, P, P).transpose(1, 0, 2).reshape(P, C * P)
        ))

    # gidx: per-call wrapped int16 local indices (idx j of a call lands at
    # [j%16, call_gcol + j//16], replicated 8x across partition groups)
    cid_blk0 = np.zeros((T, NRG), np.int64)
    cid_gcol = np.zeros((T, NRG), np.int64)
    for s in sups:
        for cl in s["calls"]:
            for t in s["tiles"]:
                cid_blk0[t, cl["r"]] = cl["blk0"]
                cid_gcol[t, cl["r"]] = cl["gcol"]
    loc_idx = (src_pad[order] - e_r * cfg.rs).astype(np.int64)
    j = (blk - cid_blk0[e_t, e_r]) * P + part
    gidx16 = np.zeros((NC, 16, gcol), np.int16)
    gidx16[e_core, j % 16, cid_gcol[e_t, e_r] + j // 16] = loc_idx
    gidx = np.tile(gidx16, (1, 8, 1))

    # ---- labels: sort by (range(el0), range(el1)) into NRG^2 groups ----
    el = edge_label_index.astype(np.int64)
    el0 = _pad_ids(cfg, el[0])
    el1 = _pad_ids(cfg, el[1])
    lpc = cfg.lab_per_core
    NG = NRG * NRG
    lab_ids = []      # per core: original label index per slot (-1 pad)
    lcounts = np.zeros((NC, NG), np.int64)
    per_core = []
    for q in range(NC):
        lo, hi = q * lpc, min((q + 1) * lpc, cfg.n_labels)
        ids = np.arange(lo, hi)
        g = (el0[ids] // cfg.rs) * NRG + (el1[ids] // cfg.rs)
        o = np.argsort(g, kind="stable")
        per_core.append((ids[o], g[o]))
        lcounts[q] = np.bincount(g, minlength=NG)
    Lg = -(-lcounts.max(axis=0) // P)  # [NG] blocks, max over cores
    LCP = int(Lg.sum())
    g_blk0 = np.zeros(NG + 1, np.int64)
    g_blk0[1:] = np.cumsum(Lg)

    l0flat = np.zeros((NC, LCP * P), np.int64)
    l1flat = np.zeros((NC, LCP * P), np.int64)
    ids_slot = -np.ones((NC, LCP * P), np.int64)
    for q in range(NC):
        ids_o, g_o = per_core[q]
        gstart = np.zeros(NG + 1, np.int64)
        gstart[1:] = np.cumsum(lcounts[q])
        posl = np.arange(len(ids_o)) - gstart[g_o]
        slot = g_blk0[g_o] * P + posl
        l0flat[q, slot] = el0[ids_o] - (g_o // NRG) * cfg.rs
        l1flat[q, slot] = el1[ids_o] - (g_o % NRG) * cfg.rs
        ids_slot[q, slot] = ids_o
    l0idx = np.stack([_wrap16(l0flat[q]) for q in range(NC)])
    l1idx = np.stack([_wrap16(l1flat[q]) for q in range(NC)])

    # label gather calls: el0 side = NRG calls (groups r0*NRG..r0*NRG+NRG-1),
    # el1 side = NG calls
    lcalls0 = []
    for r0 in range(NRG):
        nb = int(Lg[r0 * NRG: (r0 + 1) * NRG].sum())
        if nb:
            lcalls0.append(dict(r=r0, nb=nb, blk0=int(g_blk0[r0 * NRG])))
    lcalls1 = []
    for g in range(NG):
        nb = int(Lg[g])
        if nb:
            lcalls1.append(dict(r=g % NRG, nb=nb, blk0=int(g_blk0[g])))

    # node features: padded, transposed, own-shard sliced per core
    x_pad = np.zeros((cfg.n_pad, P), np.float32)
    x_pad[_pad_ids(cfg, np.arange(n))] = x
    xT = np.ascontiguousarray(x_pad.T).astype(ml_dtypes.bfloat16)

    return dict(wew=wew, gidx=gidx, l0idx=l0idx, l1idx=l1idx, xT=xT,
                sups=sups, C=C, LCP=LCP, lcalls0=lcalls0, lcalls1=lcalls1,
                ids_slot=ids_slot)


# ------------------------------------------------------------- bass program


def build_program(cfg, prep, linb_sum, phase=99):
    T, NRG = cfg.tiles_per_core, cfg.nrg
    NPAD, NLOC = cfg.n_pad, cfg.n_loc
    sups, C, LCP = prep["sups"], prep["C"], prep["LCP"]
    lcalls0, lcalls1 = prep["lcalls0"], prep["lcalls1"]
    GCOL = prep["gidx"].shape[2]
    BMAX = max(s["nblk"] for s in sups)
    rg = [list(range(NC))]

    def rrows(r):
        return min(cfg.rs, NPAD - r * cfg.rs)

    nc = bacc.Bacc(None, target_bir_lowering=False, debug=False)

    xT_d = nc.declare_dram_parameter("xT", [P, NLOC], BF, False)
    wew_d = nc.declare_dram_parameter("wew", [P, C * P], BF, False)
    gidx_d = nc.declare_dram_parameter("gidx", [P, GCOL], I16, False)
    l0_d = nc.declare_dram_parameter("l0idx", [P, LCP * 8], I16, False)
    l1_d = nc.declare_dram_parameter("l1idx", [P, LCP * 8], I16, False)
    ident_d = nc.declare_dram_parameter("ident", [P, P], BF, False)
    w1_d = nc.declare_dram_parameter("w1", [P, P], BF, False)
    w2_d = nc.declare_dram_parameter("w2", [P, P], BF, False)
    b1_d = nc.declare_dram_parameter("b1bc", [P, P], F32, False)
    b2_d = nc.declare_dram_parameter("b2bc", [P, P], F32, False)
    wv_d = nc.declare_dram_parameter("wvbc", [P, P], BF, False)
    res_d = nc.declare_dram_parameter("res", [P, LCP], F32, True)

    t1_sh = nc.dram_tensor("t1_sh", [NLOC, P], BF)
    t1_ag = nc.dram_tensor("t1_ag", [NPAD, P], BF)
    t2_sh = nc.dram_tensor("t2_sh", [NLOC, P], BF)
    t2_ag = nc.dram_tensor("t2_ag", [NPAD, P], BF)
    o2_sh = nc.dram_tensor("o2_sh", [NLOC, P], BF)
    o2_ag = nc.dram_tensor("o2_ag", [NPAD, P], BF)

    AF = mybir.ActivationFunctionType
    OP = mybir.AluOpType

    with TileContext(nc) as tc:
        with (
            tc.tile_pool(name="const", bufs=1) as cp,
            tc.tile_pool(name="wslab", bufs=2) as wp,
            tc.tile_pool(name="gbuf", bufs=2) as gp,
            tc.tile_pool(name="idx", bufs=2) as ip,
            tc.tile_pool(name="xtile", bufs=3) as xp,
            tc.tile_pool(name="evict", bufs=4) as ep,
            tc.tile_pool(name="lab", bufs=2) as lp,
            tc.tile_pool(name="ps_deg", bufs=1, space="PSUM") as psd,
            tc.tile_pool(name="ps_agg", bufs=2, space="PSUM") as psa,
            tc.tile_pool(name="ps_gem", bufs=2, space="PSUM") as psg,
            tc.tile_pool(name="ps_tr", bufs=1, space="PSUM") as pst,
        ):
            # ---- persistent SBUF ----
            ident_sb = cp.tile([P, P], BF)
            nc.sync.dma_start(out=ident_sb[:], in_=ident_d[:, :])
            w1_sb = cp.tile([P, P], BF)
            nc.sync.dma_start(out=w1_sb[:], in_=w1_d[:, :])
            w2_sb = cp.tile([P, P], BF)
            nc.sync.dma_start(out=w2_sb[:], in_=w2_d[:, :])
            b1_sb = cp.tile([P, P], F32)
            nc.sync.dma_start(out=b1_sb[:], in_=b1_d[:, :])
            b2_sb = cp.tile([P, P], F32)
            nc.sync.dma_start(out=b2_sb[:], in_=b2_d[:, :])
            wv_sb = cp.tile([P, P], BF)
            nc.sync.dma_start(out=wv_sb[:], in_=wv_d[:, :])
            l0_sb = cp.tile([P, LCP * 8], I16)
            nc.sync.dma_start(out=l0_sb[:], in_=l0_d[:, :])
            l1_sb = cp.tile([P, LCP * 8], I16)
            nc.sync.dma_start(out=l1_sb[:], in_=l1_d[:, :])
            ones_sb = cp.tile([P, 1], BF)
            nc.vector.memset(ones_sb[:], 1.0)
            deg_sb = cp.tile([P, T], F32)
            rec_sb = cp.tile([P, T], F32)
            dinv_own = cp.tile([P, T], F32)
            res_sb = cp.tile([P, LCP], F32)

            def load_wslab(s):
                w = wp.tile([P, BMAX * P], BF, tag="w")
                nb = s["nblk"]
                c0 = s["slab0"] * P
                nc.sync.dma_start(
                    out=w[:, : nb * P], in_=wew_d[:, c0: c0 + nb * P]
                )
                return w

            # ---- deg pass (own tiles) ----
            for s in sups:
                w = load_wslab(s)
                for t in s["tiles"]:
                    blks = s["tile_chunks"][t]
                    pd = psd.tile([P, 1], F32)
                    for i, b in enumerate(blks):
                        nc.tensor.matmul(
                            out=pd[:],
                            lhsT=w[:, b * P: (b + 1) * P],
                            rhs=ones_sb[:],
                            start=(i == 0),
                            stop=(i == len(blks) - 1),
                        )
                    nc.scalar.activation(deg_sb[:, t: t + 1], pd[:], AF.Copy)
            nc.vector.tensor_scalar_max(deg_sb[:], deg_sb[:], 1.0)
            nc.vector.reciprocal(rec_sb[:], deg_sb[:])
            nc.scalar.activation(dinv_own[:], rec_sb[:], AF.Sqrt)

            # ---- layer-1 table: own shard of dinv*(x@W1), then AllGather
            for t in range(T):
                lhsT = xp.tile([P, P], BF, tag="lhsT")
                nc.sync.dma_start(
                    out=lhsT[:], in_=xT_d[:, t * P: (t + 1) * P]
                )
                pg = psg.tile([P, P], F32, tag="pg")
                nc.tensor.matmul(
                    out=pg[:], lhsT=lhsT[:], rhs=w1_sb[:],
                    start=True, stop=True,
                )
                hbf = xp.tile([P, P], BF, tag="hbf")
                nc.scalar.activation(
                    hbf[:], pg[:], AF.Copy, scale=dinv_own[:, t: t + 1]
                )
                nc.sync.dma_start(
                    out=t1_sh[t * P: (t + 1) * P, :], in_=hbf[:]
                )
            nc.gpsimd.collective_compute(
                "AllGather", OP.bypass, replica_groups=rg,
                ins=[t1_sh[:, :]], outs=[t1_ag[:, :]],
            )
            if phase == 1:
                pr = cp.tile([P, LCP], F32)
                nc.vector.tensor_copy(pr[:], dinv_own[:, :1].to_broadcast([P, LCP]))
                nc.sync.dma_start(out=res_d[:, :], in_=pr[:])

            # ---- aggregation supertile machinery ----
            def gather_sup(s, table):
                g = gp.tile([P, BMAX * P], BF, tag="g")
                for cl in s["calls"]:
                    r, nb = cl["r"], cl["nb"]
                    b0 = cl["blk0"] - s["slab0"]
                    it = ip.tile([P, BMAX * 8], I16, tag="gi")
                    nc.sync.dma_start(
                        out=it[:, : nb * 8],
                        in_=gidx_d[:, cl["gcol"]: cl["gcol"] + nb * 8],
                    )
                    for q0 in range(0, nb, GMAX):
                        qn = min(GMAX, nb - q0)
                        nc.gpsimd.dma_gather(
                            g[:, (b0 + q0) * P: (b0 + q0 + qn) * P].rearrange(
                                "p (g e) -> p g e", e=P
                            ),
                            table[r * cfg.rs: r * cfg.rs + rrows(r), :],
                            it[:, q0 * 8: (q0 + qn) * 8],
                            qn * P,
                            qn * P,
                            P,
                        )
                return g

            # ---- layer 1: aggregate t1 -> out1, emit t2 = dinv*out1 ----
            def agg1():
                for s in sups:
                    w = load_wslab(s)
                    g = gather_sup(s, t1_ag)
                    for t in s["tiles"]:
                        blks = s["tile_chunks"][t]
                        pa = psa.tile([P, P], F32)
                        for i, b in enumerate(blks):
                            sl = slice(b * P, (b + 1) * P)
                            nc.tensor.matmul(
                                out=pa[:], lhsT=w[:, sl], rhs=g[:, sl],
                                start=(i == 0), stop=(i == len(blks) - 1),
                            )
                        t1 = ep.tile([P, P], F32, tag="t1")
                        nc.scalar.activation(
                            t1[:], pa[:], AF.Copy,
                            scale=dinv_own[:, t: t + 1],
                        )
                        nc.vector.tensor_tensor(
                            out=t1[:], in0=t1[:], in1=b1_sb[:], op=OP.add
                        )
                        o1 = ep.tile([P, P], F32, tag="o1")
                        nc.scalar.activation(o1[:], t1[:], AF.Relu)
                        t2b = ep.tile([P, P], BF, tag="t2b")
                        nc.scalar.activation(
                            t2b[:], o1[:], AF.Copy,
                            scale=dinv_own[:, t: t + 1],
                        )
                        nc.sync.dma_start(
                            out=t2_sh[t * P: (t + 1) * P, :], in_=t2b[:]
                        )

            # ---- layer 2: aggregate t2 (f-major psum), GEMM W2, evict ----
            def agg2():
                for s in sups:
                    w = load_wslab(s)
                    g = gather_sup(s, t2_ag)
                    for t in s["tiles"]:
                        blks = s["tile_chunks"][t]
                        pa = psa.tile([P, P], F32)
                        for i, b in enumerate(blks):
                            sl = slice(b * P, (b + 1) * P)
                            nc.tensor.matmul(
                                out=pa[:], lhsT=g[:, sl], rhs=w[:, sl],
                                start=(i == 0), stop=(i == len(blks) - 1),
                            )
                        uT = ep.tile([P, P], BF, tag="uT")
                        nc.scalar.activation(uT[:], pa[:], AF.Copy)
                        pb = psg.tile([P, P], F32, tag="pg")
                        nc.tensor.matmul(
                            out=pb[:], lhsT=w2_sb[:], rhs=uT[:],
                            start=True, stop=True,
                        )
                        vT = ep.tile([P, P], BF, tag="vT")
                        nc.scalar.activation(vT[:], pb[:], AF.Copy)
                        pt = pst.tile([P, P], BF)
                        nc.tensor.transpose(
                            out=pt[:], in_=vT[:], identity=ident_sb[:]
                        )
                        t1 = ep.tile([P, P], F32, tag="t1")
                        nc.scalar.activation(
                            t1[:], pt[:], AF.Copy,
                            scale=dinv_own[:, t: t + 1],
                        )
                        nc.vector.tensor_tensor(
                            out=t1[:], in0=t1[:], in1=b2_sb[:], op=OP.add
                        )
                        o2 = ep.tile([P, P], BF, tag="o2")
                        nc.scalar.activation(o2[:], t1[:], AF.Relu)
                        nc.sync.dma_start(
                            out=o2_sh[t * P: (t + 1) * P, :], in_=o2[:]
                        )

            def probe(src):
                pr = cp.tile([P, P], BF)
                nc.sync.dma_start(out=pr[:], in_=src)
                prf = cp.tile([P, P], F32)
                nc.vector.tensor_copy(prf[:], pr[:])
                nc.sync.dma_start(out=res_d[:, : min(LCP, P)],
                                  in_=prf[:, : min(LCP, P)])

            if phase >= 2:
                agg1()
            if phase == 2:
                probe(t2_sh[0:P, :])
            if phase >= 3:
                nc.gpsimd.collective_compute(
                    "AllGather", OP.bypass, replica_groups=rg,
                    ins=[t2_sh[:, :]], outs=[t2_ag[:, :]],
                )
            if phase == 3:
                probe(t2_ag[0:P, :])
            if phase >= 4:
                agg2()
                nc.gpsimd.collective_compute(
                    "AllGather", OP.bypass, replica_groups=rg,
                    ins=[o2_sh[:, :]], outs=[o2_ag[:, :]],
                )
            if phase == 4:
                probe(o2_ag[0:P, :])

            # ---- label pass: process one el0-range region at a time ----
            LB = max((cl["nb"] for cl in lcalls0), default=1)

            def lab_gather_region(cl0):
                b00, nb0 = cl0["blk0"], cl0["nb"]
                ga = lp.tile([P, LB * P], BF, tag="ga")
                for q0 in range(0, nb0, GMAX):
                    qn = min(GMAX, nb0 - q0)
                    nc.gpsimd.dma_gather(
                        ga[:, q0 * P: (q0 + qn) * P].rearrange(
                            "p (g e) -> p g e", e=P
                        ),
                        o2_ag[cl0["r"] * cfg.rs: cl0["r"] * cfg.rs
                              + rrows(cl0["r"]), :],
                        l0_sb[:, (b00 + q0) * 8: (b00 + q0 + qn) * 8],
                        qn * P,
                        qn * P,
                        P,
                    )
                gb = lp.tile([P, LB * P], BF, tag="gb")
                for cl in lcalls1:
                    if not (b00 <= cl["blk0"] < b00 + nb0):
                        continue
                    r, nb = cl["r"], cl["nb"]
                    o = cl["blk0"] - b00
                    for q0 in range(0, nb, GMAX):
                        qn = min(GMAX, nb - q0)
                        nc.gpsimd.dma_gather(
                            gb[:, (o + q0) * P: (o + q0 + qn) * P].rearrange(
                                "p (g e) -> p g e", e=P
                            ),
                            o2_ag[r * cfg.rs: r * cfg.rs + rrows(r), :],
                            l1_sb[:, (cl["blk0"] + q0) * 8:
                                  (cl["blk0"] + q0 + qn) * 8],
                            qn * P,
                            qn * P,
                            P,
                        )
                return ga, gb

            if phase >= 5:
                for cl0 in lcalls0:
                    ga, gb = lab_gather_region(cl0)
                    for ci in range(cl0["nb"]):
                        c = cl0["blk0"] + ci
                        sl = slice(ci * P, (ci + 1) * P)
                        prod = lp.tile([P, P], BF, tag="prod")
                        nc.vector.tensor_tensor(
                            out=prod[:], in0=ga[:, sl], in1=gb[:, sl],
                            op=OP.mult,
                        )
                        scr = lp.tile([P, P], BF, tag="scr")
                        nc.vector.tensor_tensor(
                            out=scr[:], in0=prod[:], in1=wv_sb[:], op=OP.mult
                        )
                        nc.vector.reduce_sum(
                            res_sb[:, c: c + 1], scr[:],
                            axis=mybir.AxisListType.X,
                        )
                nc.vector.tensor_scalar_add(
                    res_sb[:], res_sb[:], float(linb_sum)
                )
                nc.sync.dma_start(out=res_d[:, :], in_=res_sb[:])

    nc.finalize()
    return nc


# ------------------------------------------------------------------ driver


def make_in_maps(cfg, prep, W1, b1, W2, b2, lin_W, lin_b):
    consts = dict(
        ident=np.eye(P, dtype=np.float32).astype(ml_dtypes.bfloat16),
        w1=W1.astype(np.float32).astype(ml_dtypes.bfloat16),
        w2=W2.astype(np.float32).astype(ml_dtypes.bfloat16),
        b1bc=np.tile(b1.astype(np.float32)[None, :], (P, 1)),
        b2bc=np.tile(b2.astype(np.float32)[None, :], (P, 1)),
        wvbc=np.tile(
            lin_W.astype(np.float32).sum(axis=1)[None, :], (P, 1)
        ).astype(ml_dtypes.bfloat16),
    )
    in_maps = []
    for q in range(NC):
        m = dict(consts)
        m.update(
            xT=np.ascontiguousarray(
                prep["xT"][:, q * cfg.n_loc: (q + 1) * cfg.n_loc]
            ),
            wew=prep["wew"][q],
            gidx=prep["gidx"][q],
            l0idx=prep["l0idx"][q],
            l1idx=prep["l1idx"][q],
        )
        in_maps.append(m)
    return in_maps


def assemble_output(cfg, prep, results):
    out = np.zeros(cfg.n_labels, np.float32)
    for q in range(NC):
        r = np.asarray(results[q]["res"], np.float32)  # [128, LCP]
        flat = r.T.reshape(-1)  # slot j = c*128+p -> [c, p] row-major
        ids = prep["ids_slot"][q]
        m = ids >= 0
        out[ids[m]] = flat[m]
    return out


def run(cfg, x, edge_index, edge_weight, edge_label_index,
        W1, b1, W2, b2, lin_W, lin_b, trace=False, phase=99):
    global LAST_EXEC_NS, LAST_RESULTS
    prep = preprocess(cfg, np.asarray(x), np.asarray(edge_index),
                      np.asarray(edge_weight), np.asarray(edge_label_index))
    linb_sum = float(np.asarray(lin_b, np.float64).sum())
    nc = build_program(cfg, prep, linb_sum, phase=phase)
    in_maps = make_in_maps(cfg, prep, W1, b1, W2, b2, lin_W, lin_b)
    res = run_bass_kernel_spmd(
        nc, in_maps, list(range(NC)), trace=trace
    )
    LAST_EXEC_NS = res.exec_time_ns
    LAST_RESULTS = res
    return assemble_output(cfg, prep, res.results)


def kernel(x, edge_index, edge_weight, edge_label_index,
           W1, b1, W2, b2, lin_W, lin_b):
    trace = bool(os.environ.get("KERNEL_TRACE"))
    return run(FULL, x, edge_index, edge_weight, edge_label_index,
               W1, b1, W2, b2, lin_W, lin_b, trace=trace)


# revision 21
# speedup vs baseline: 1.5297x; 1.1760x over previous
"""GCN link-predictor kernel for 8 Trainium2 NeuronCores (Bass/Tile).

Strategy (SPMD, dst-sharded, v2):
  - Host: append self loops, assign each edge to the core owning its dst,
    group per 128-node dst tile, sort each tile's edges by src range
    (32768 rows = int16 dma_gather window), pad each (tile, range) to
    whole 128-edge chunks.  Ship the one-hot scatter matrices W_ew
    (W[e, dstl] = ew) pre-built in bf16, plus int16 local gather indices
    (16-partition wrapped) -- so the device does no one-hot construction.
  - Device per layer:  gather table rows with dma_gather (one call per
    supertile x src-range; ~1us SWDGE overhead amortized over thousands
    of rows), then chunk matmuls vs W_ew accumulate dst-tile partials in
    PSUM.  Layer1 table = dinv (.) (x @ W1) built from the core's own
    shard (lhsT = host-pre-transposed xT) and AllGathered; layer2 table
    = dinv (.) out1 produced directly at the layer1 evict (aggregate-
    then-GEMM reordering: (A X) W = A (X W)), so no full-table GEMM pass
    and no global dinv exchange -- all dinv folds are own-shard.
  - Labels: pairs sorted by (range(el0), range(el1)) into 16 groups so
    both sides gather via dma_gather; score = sum(a*b*wv) + sum(lin_b)
    with wv = lin_W @ 1.  Host un-permutes the result.
"""

import os
import sys

import numpy as np

for _p in ("/opt/trn_rl_repo",):
    if _p not in sys.path:
        sys.path.insert(0, _p)

import ml_dtypes  # noqa: E402

import concourse.bacc as bacc  # noqa: E402
import concourse.bass as bass  # noqa: E402
import concourse.mybir as mybir  # noqa: E402
from concourse.bass_utils import run_bass_kernel_spmd  # noqa: E402
from concourse.tile import TileContext  # noqa: E402

P = 128
NC = 8
RS = 32768  # dma_gather int16 window (rows)
SUP = 4    # dst tiles per supertile (gather-call granularity)
GMAX = 8   # max 128-row blocks per dma_gather call (SWDGE ring limit)
BF = mybir.dt.bfloat16
F32 = mybir.dt.float32
I16 = mybir.dt.int16

LAST_EXEC_NS = None
LAST_RESULTS = None


class Cfg:
    def __init__(self, n_nodes, n_labels, rs=RS):
        assert n_nodes % NC == 0
        self.n_nodes = n_nodes
        self.nodes_per_core = n_nodes // NC
        self.tiles_per_core = -(-self.nodes_per_core // P)
        self.n_loc = self.tiles_per_core * P
        self.n_pad = NC * self.n_loc
        self.n_labels = n_labels
        self.lab_per_core = -(-n_labels // NC)
        self.rs = rs
        self.nrg = -(-self.n_pad // rs)


FULL = Cfg(100000, 200000)


# ---------------------------------------------------------------- host prep


def _pad_ids(cfg, ids):
    q = np.minimum(ids // cfg.nodes_per_core, NC - 1)
    l = ids - q * cfg.nodes_per_core
    return q * cfg.n_loc + l


def _wrap16(flat_idx):
    # [n] -> [128, n//16]: idx j at [j%16, j//16], replicated to 128 parts
    n = len(flat_idx)
    assert n % 16 == 0
    a = np.zeros((16, n // 16), np.int16)
    a[np.arange(n) % 16, np.arange(n) // 16] = flat_idx
    return np.tile(a, (8, 1))


def preprocess(cfg, x, edge_index, edge_weight, edge_label_index):
    n = cfg.n_nodes
    T, NRG = cfg.tiles_per_core, cfg.nrg
    src = np.concatenate([edge_index[0], np.arange(n)]).astype(np.int64)
    dst = np.concatenate([edge_index[1], np.arange(n)]).astype(np.int64)
    ew = np.concatenate(
        [edge_weight.astype(np.float32), np.ones(n, np.float32)]
    )

    src_pad = _pad_ids(cfg, src)
    dst_pad = _pad_ids(cfg, dst)
    dq, dl = np.divmod(dst_pad, cfg.n_loc)
    dt_ = dl // P          # dst tile within core
    dloc = dl % P          # dst row within tile
    srange = src_pad // cfg.rs

    # chunk counts per (tile, range): max over cores for SPMD uniformity
    key = (dq * T + dt_) * NRG + srange
    counts = np.bincount(key, minlength=NC * T * NRG).reshape(NC, T, NRG)
    Kr = -(-counts.max(axis=0) // P)  # [T, NRG] blocks (may be 0)

    # supertile slab layout: for each supertile, range-major block order
    sups = []
    boff = np.zeros((T, NRG), np.int64)  # global block offset of (t, r)
    gcol = 0  # running gidx column offset
    nblk = 0
    for t0 in range(0, T, SUP):
        tiles = list(range(t0, min(t0 + SUP, T)))
        calls = []
        slab0 = nblk
        tile_chunks = {t: [] for t in tiles}
        for r in range(NRG):
            nb = int(sum(Kr[t, r] for t in tiles))
            if nb == 0:
                continue
            call_start = nblk
            for t in tiles:
                boff[t, r] = nblk
                tile_chunks[t].extend(range(nblk, nblk + int(Kr[t, r])))
                nblk += int(Kr[t, r])
            calls.append(dict(r=r, nb=nb, blk0=call_start, gcol=gcol))
            gcol += nb * 8  # nb*128/16 cols
        sups.append(dict(tiles=tiles, calls=calls, slab0=slab0,
                         nblk=nblk - slab0,
                         tile_chunks={t: [b - slab0 for b in tile_chunks[t]]
                                      for t in tiles}))
    C = nblk

    # slot assignment for every edge
    order = np.argsort(key, kind="stable")
    sk = key[order]
    starts = np.zeros(NC * T * NRG + 1, np.int64)
    starts[1:] = np.cumsum(counts.reshape(-1))
    pos = np.arange(len(order)) - starts[sk]
    e_core = order * 0 + dq[order]
    e_t, e_r = dt_[order], srange[order]
    blk = boff[e_t, e_r] + pos // P
    part = pos % P

    wew_lin = blk * (P * P) + part * P + dloc[order]
    ew_bf = ew[order].astype(ml_dtypes.bfloat16)
    wew = []
    for q in range(NC):
        m = e_core == q
        arr = np.zeros(C * P * P, ml_dtypes.bfloat16)
        arr[wew_lin[m]] = ew_bf[m]
        wew.append(np.ascontiguousarray(
            arr.reshape(C, P, P).transpose(1, 0, 2).reshape(P, C * P)
        ))

    # gidx: per-call wrapped int16 local indices (idx j of a call lands at
    # [j%16, call_gcol + j//16], replicated 8x across partition groups)
    cid_blk0 = np.zeros((T, NRG), np.int64)
    cid_gcol = np.zeros((T, NRG), np.int64)
    for s in sups:
        for cl in s["calls"]:
            for t in s["tiles"]:
                cid_blk0[t, cl["r"]] = cl["blk0"]
                cid_gcol[t, cl["r"]] = cl["gcol"]
    loc_idx = (src_pad[order] - e_r * cfg.rs).astype(np.int64)
    j = (blk - cid_blk0[e_t, e_r]) * P + part
    gidx16 = np.zeros((NC, 16, gcol), np.int16)
    gidx16[e_core, j % 16, cid_gcol[e_t, e_r] + j // 16] = loc_idx
    gidx = np.tile(gidx16, (1, 8, 1))

    # ---- labels: sort by (range(el0), range(el1)) into NRG^2 groups ----
    el = edge_label_index.astype(np.int64)
    el0 = _pad_ids(cfg, el[0])
    el1 = _pad_ids(cfg, el[1])
    lpc = cfg.lab_per_core
    NG = NRG * NRG
    lab_ids = []      # per core: original label index per slot (-1 pad)
    lcounts = np.zeros((NC, NG), np.int64)
    per_core = []
    for q in range(NC):
        lo, hi = q * lpc, min((q + 1) * lpc, cfg.n_labels)
        ids = np.arange(lo, hi)
        g = (el0[ids] // cfg.rs) * NRG + (el1[ids] // cfg.rs)
        o = np.argsort(g, kind="stable")
        per_core.append((ids[o], g[o]))
        lcounts[q] = np.bincount(g, minlength=NG)
    Lg = -(-lcounts.max(axis=0) // P)  # [NG] blocks, max over cores
    LCP = int(Lg.sum())
    g_blk0 = np.zeros(NG + 1, np.int64)
    g_blk0[1:] = np.cumsum(Lg)

    l0flat = np.zeros((NC, LCP * P), np.int64)
    l1flat = np.zeros((NC, LCP * P), np.int64)
    ids_slot = -np.ones((NC, LCP * P), np.int64)
    for q in range(NC):
        ids_o, g_o = per_core[q]
        gstart = np.zeros(NG + 1, np.int64)
        gstart[1:] = np.cumsum(lcounts[q])
        posl = np.arange(len(ids_o)) - gstart[g_o]
        slot = g_blk0[g_o] * P + posl
        l0flat[q, slot] = el0[ids_o] - (g_o // NRG) * cfg.rs
        l1flat[q, slot] = el1[ids_o] - (g_o % NRG) * cfg.rs
        ids_slot[q, slot] = ids_o
    l0idx = np.stack([_wrap16(l0flat[q]) for q in range(NC)])
    l1idx = np.stack([_wrap16(l1flat[q]) for q in range(NC)])

    # label gather calls: el0 side = NRG calls (groups r0*NRG..r0*NRG+NRG-1),
    # el1 side = NG calls
    lcalls0 = []
    for r0 in range(NRG):
        nb = int(Lg[r0 * NRG: (r0 + 1) * NRG].sum())
        if nb:
            lcalls0.append(dict(r=r0, nb=nb, blk0=int(g_blk0[r0 * NRG])))
    lcalls1 = []
    for g in range(NG):
        nb = int(Lg[g])
        if nb:
            lcalls1.append(dict(r=g % NRG, nb=nb, blk0=int(g_blk0[g])))

    # node features: padded, transposed, own-shard sliced per core
    x_pad = np.zeros((cfg.n_pad, P), np.float32)
    x_pad[_pad_ids(cfg, np.arange(n))] = x
    xT = np.ascontiguousarray(x_pad.T).astype(ml_dtypes.bfloat16)

    return dict(wew=wew, gidx=gidx, l0idx=l0idx, l1idx=l1idx, xT=xT,
                sups=sups, C=C, LCP=LCP, lcalls0=lcalls0, lcalls1=lcalls1,
                ids_slot=ids_slot)


# ------------------------------------------------------------- bass program


def build_program(cfg, prep, linb_sum, phase=99):
    T, NRG = cfg.tiles_per_core, cfg.nrg
    NPAD, NLOC = cfg.n_pad, cfg.n_loc
    sups, C, LCP = prep["sups"], prep["C"], prep["LCP"]
    lcalls0, lcalls1 = prep["lcalls0"], prep["lcalls1"]
    GCOL = prep["gidx"].shape[2]
    BMAX = max(s["nblk"] for s in sups)
    rg = [list(range(NC))]

    def rrows(r):
        return min(cfg.rs, NPAD - r * cfg.rs)

    nc = bacc.Bacc(None, target_bir_lowering=False, debug=False,
                   num_swdge_queues=4)

    xT_d = nc.declare_dram_parameter("xT", [P, NLOC], BF, False)
    wew_d = nc.declare_dram_parameter("wew", [P, C * P], BF, False)
    gidx_d = nc.declare_dram_parameter("gidx", [P, GCOL], I16, False)
    l0_d = nc.declare_dram_parameter("l0idx", [P, LCP * 8], I16, False)
    l1_d = nc.declare_dram_parameter("l1idx", [P, LCP * 8], I16, False)
    ident_d = nc.declare_dram_parameter("ident", [P, P], BF, False)
    w1_d = nc.declare_dram_parameter("w1", [P, P], BF, False)
    w2_d = nc.declare_dram_parameter("w2", [P, P], BF, False)
    b1_d = nc.declare_dram_parameter("b1bc", [P, P], F32, False)
    b2_d = nc.declare_dram_parameter("b2bc", [P, P], F32, False)
    wv_d = nc.declare_dram_parameter("wvbc", [P, P], BF, False)
    res_d = nc.declare_dram_parameter("res", [P, LCP], F32, True)

    t1_sh = nc.dram_tensor("t1_sh", [NLOC, P], BF)
    t1_ag = nc.dram_tensor("t1_ag", [NPAD, P], BF)
    t2_sh = nc.dram_tensor("t2_sh", [NLOC, P], BF)
    t2_ag = nc.dram_tensor("t2_ag", [NPAD, P], BF)
    o2_sh = nc.dram_tensor("o2_sh", [NLOC, P], BF)
    o2_ag = nc.dram_tensor("o2_ag", [NPAD, P], BF)

    AF = mybir.ActivationFunctionType
    OP = mybir.AluOpType
    qctr = [0]

    def next_q():
        qctr[0] = (qctr[0] + 1) % 4
        return qctr[0]

    with TileContext(nc) as tc:
        with (
            tc.tile_pool(name="const", bufs=1) as cp,
            tc.tile_pool(name="wslab", bufs=2) as wp,
            tc.tile_pool(name="gbuf", bufs=2) as gp,
            tc.tile_pool(name="idx", bufs=2) as ip,
            tc.tile_pool(name="xtile", bufs=3) as xp,
            tc.tile_pool(name="evict", bufs=4) as ep,
            tc.tile_pool(name="lab", bufs=2) as lp,
            tc.tile_pool(name="ps_deg", bufs=1, space="PSUM") as psd,
            tc.tile_pool(name="ps_agg", bufs=2, space="PSUM") as psa,
            tc.tile_pool(name="ps_gem", bufs=2, space="PSUM") as psg,
            tc.tile_pool(name="ps_tr", bufs=1, space="PSUM") as pst,
        ):
            # ---- persistent SBUF ----
            ident_sb = cp.tile([P, P], BF)
            nc.sync.dma_start(out=ident_sb[:], in_=ident_d[:, :])
            w1_sb = cp.tile([P, P], BF)
            nc.sync.dma_start(out=w1_sb[:], in_=w1_d[:, :])
            w2_sb = cp.tile([P, P], BF)
            nc.sync.dma_start(out=w2_sb[:], in_=w2_d[:, :])
            b1_sb = cp.tile([P, P], F32)
            nc.sync.dma_start(out=b1_sb[:], in_=b1_d[:, :])
            b2_sb = cp.tile([P, P], F32)
            nc.sync.dma_start(out=b2_sb[:], in_=b2_d[:, :])
            wv_sb = cp.tile([P, P], BF)
            nc.sync.dma_start(out=wv_sb[:], in_=wv_d[:, :])
            l0_sb = cp.tile([P, LCP * 8], I16)
            nc.sync.dma_start(out=l0_sb[:], in_=l0_d[:, :])
            l1_sb = cp.tile([P, LCP * 8], I16)
            nc.sync.dma_start(out=l1_sb[:], in_=l1_d[:, :])
            ones_sb = cp.tile([P, 1], BF)
            nc.vector.memset(ones_sb[:], 1.0)
            deg_sb = cp.tile([P, T], F32)
            rec_sb = cp.tile([P, T], F32)
            dinv_own = cp.tile([P, T], F32)
            res_sb = cp.tile([P, LCP], F32)

            def load_wslab(s):
                w = wp.tile([P, BMAX * P], BF, tag="w")
                nb = s["nblk"]
                c0 = s["slab0"] * P
                nc.sync.dma_start(
                    out=w[:, : nb * P], in_=wew_d[:, c0: c0 + nb * P]
                )
                return w

            # ---- deg pass (own tiles) ----
            for s in sups:
                w = load_wslab(s)
                for t in s["tiles"]:
                    blks = s["tile_chunks"][t]
                    pd = psd.tile([P, 1], F32)
                    for i, b in enumerate(blks):
                        nc.tensor.matmul(
                            out=pd[:],
                            lhsT=w[:, b * P: (b + 1) * P],
                            rhs=ones_sb[:],
                            start=(i == 0),
                            stop=(i == len(blks) - 1),
                        )
                    nc.scalar.activation(deg_sb[:, t: t + 1], pd[:], AF.Copy)
            nc.vector.tensor_scalar_max(deg_sb[:], deg_sb[:], 1.0)
            nc.vector.reciprocal(rec_sb[:], deg_sb[:])
            nc.scalar.activation(dinv_own[:], rec_sb[:], AF.Sqrt)

            # ---- layer-1 table: own shard of dinv*(x@W1), then AllGather
            for t in range(T):
                lhsT = xp.tile([P, P], BF, tag="lhsT")
                nc.sync.dma_start(
                    out=lhsT[:], in_=xT_d[:, t * P: (t + 1) * P]
                )
                pg = psg.tile([P, P], F32, tag="pg")
                nc.tensor.matmul(
                    out=pg[:], lhsT=lhsT[:], rhs=w1_sb[:],
                    start=True, stop=True,
                )
                hbf = xp.tile([P, P], BF, tag="hbf")
                nc.scalar.activation(
                    hbf[:], pg[:], AF.Copy, scale=dinv_own[:, t: t + 1]
                )
                nc.sync.dma_start(
                    out=t1_sh[t * P: (t + 1) * P, :], in_=hbf[:]
                )
            nc.gpsimd.collective_compute(
                "AllGather", OP.bypass, replica_groups=rg,
                ins=[t1_sh[:, :]], outs=[t1_ag[:, :]],
            )
            if phase == 1:
                pr = cp.tile([P, LCP], F32)
                nc.vector.tensor_copy(pr[:], dinv_own[:, :1].to_broadcast([P, LCP]))
                nc.sync.dma_start(out=res_d[:, :], in_=pr[:])

            # ---- aggregation supertile machinery ----
            def gather_sup(s, table):
                g = gp.tile([P, BMAX * P], BF, tag="g")
                for cl in s["calls"]:
                    r, nb = cl["r"], cl["nb"]
                    b0 = cl["blk0"] - s["slab0"]
                    it = ip.tile([P, BMAX * 8], I16, tag="gi")
                    nc.sync.dma_start(
                        out=it[:, : nb * 8],
                        in_=gidx_d[:, cl["gcol"]: cl["gcol"] + nb * 8],
                    )
                    for q0 in range(0, nb, GMAX):
                        qn = min(GMAX, nb - q0)
                        nc.gpsimd.dma_gather(
                            g[:, (b0 + q0) * P: (b0 + q0 + qn) * P].rearrange(
                                "p (g e) -> p g e", e=P
                            ),
                            table[r * cfg.rs: r * cfg.rs + rrows(r), :],
                            it[:, q0 * 8: (q0 + qn) * 8],
                            qn * P,
                            qn * P,
                            P,
                            queue_num=next_q(),
                        )
                return g

            # ---- layer 1: aggregate t1 -> out1, emit t2 = dinv*out1 ----
            def agg1():
                for s in sups:
                    w = load_wslab(s)
                    g = gather_sup(s, t1_ag)
                    for t in s["tiles"]:
                        blks = s["tile_chunks"][t]
                        pa = psa.tile([P, P], F32)
                        for i, b in enumerate(blks):
                            sl = slice(b * P, (b + 1) * P)
                            nc.tensor.matmul(
                                out=pa[:], lhsT=w[:, sl], rhs=g[:, sl],
                                start=(i == 0), stop=(i == len(blks) - 1),
                            )
                        t1 = ep.tile([P, P], F32, tag="t1")
                        nc.scalar.activation(
                            t1[:], pa[:], AF.Copy,
                            scale=dinv_own[:, t: t + 1],
                        )
                        nc.vector.tensor_tensor(
                            out=t1[:], in0=t1[:], in1=b1_sb[:], op=OP.add
                        )
                        o1 = ep.tile([P, P], F32, tag="o1")
                        nc.scalar.activation(o1[:], t1[:], AF.Relu)
                        t2b = ep.tile([P, P], BF, tag="t2b")
                        nc.scalar.activation(
                            t2b[:], o1[:], AF.Copy,
                            scale=dinv_own[:, t: t + 1],
                        )
                        nc.sync.dma_start(
                            out=t2_sh[t * P: (t + 1) * P, :], in_=t2b[:]
                        )

            # ---- layer 2: aggregate t2 (f-major psum), GEMM W2, evict ----
            def agg2():
                for s in sups:
                    w = load_wslab(s)
                    g = gather_sup(s, t2_ag)
                    for t in s["tiles"]:
                        blks = s["tile_chunks"][t]
                        pa = psa.tile([P, P], F32)
                        for i, b in enumerate(blks):
                            sl = slice(b * P, (b + 1) * P)
                            nc.tensor.matmul(
                                out=pa[:], lhsT=g[:, sl], rhs=w[:, sl],
                                start=(i == 0), stop=(i == len(blks) - 1),
                            )
                        uT = ep.tile([P, P], BF, tag="uT")
                        nc.scalar.activation(uT[:], pa[:], AF.Copy)
                        pb = psg.tile([P, P], F32, tag="pg")
                        nc.tensor.matmul(
                            out=pb[:], lhsT=w2_sb[:], rhs=uT[:],
                            start=True, stop=True,
                        )
                        vT = ep.tile([P, P], BF, tag="vT")
                        nc.scalar.activation(vT[:], pb[:], AF.Copy)
                        pt = pst.tile([P, P], BF)
                        nc.tensor.transpose(
                            out=pt[:], in_=vT[:], identity=ident_sb[:]
                        )
                        t1 = ep.tile([P, P], F32, tag="t1")
                        nc.scalar.activation(
                            t1[:], pt[:], AF.Copy,
                            scale=dinv_own[:, t: t + 1],
                        )
                        nc.vector.tensor_tensor(
                            out=t1[:], in0=t1[:], in1=b2_sb[:], op=OP.add
                        )
                        o2 = ep.tile([P, P], BF, tag="o2")
                        nc.scalar.activation(o2[:], t1[:], AF.Relu)
                        nc.sync.dma_start(
                            out=o2_sh[t * P: (t + 1) * P, :], in_=o2[:]
                        )

            def probe(src):
                pr = cp.tile([P, P], BF)
                nc.sync.dma_start(out=pr[:], in_=src)
                prf = cp.tile([P, P], F32)
                nc.vector.tensor_copy(prf[:], pr[:])
                nc.sync.dma_start(out=res_d[:, : min(LCP, P)],
                                  in_=prf[:, : min(LCP, P)])

            if phase >= 2:
                agg1()
            if phase == 2:
                probe(t2_sh[0:P, :])
            if phase >= 3:
                nc.gpsimd.collective_compute(
                    "AllGather", OP.bypass, replica_groups=rg,
                    ins=[t2_sh[:, :]], outs=[t2_ag[:, :]],
                )
            if phase == 3:
                probe(t2_ag[0:P, :])
            if phase >= 4:
                agg2()
                nc.gpsimd.collective_compute(
                    "AllGather", OP.bypass, replica_groups=rg,
                    ins=[o2_sh[:, :]], outs=[o2_ag[:, :]],
                )
            if phase == 4:
                probe(o2_ag[0:P, :])

            # ---- label pass: process one el0-range region at a time ----
            LB = max((cl["nb"] for cl in lcalls0), default=1)

            def lab_gather_region(cl0):
                b00, nb0 = cl0["blk0"], cl0["nb"]
                ga = lp.tile([P, LB * P], BF, tag="ga")
                for q0 in range(0, nb0, GMAX):
                    qn = min(GMAX, nb0 - q0)
                    nc.gpsimd.dma_gather(
                        ga[:, q0 * P: (q0 + qn) * P].rearrange(
                            "p (g e) -> p g e", e=P
                        ),
                        o2_ag[cl0["r"] * cfg.rs: cl0["r"] * cfg.rs
                              + rrows(cl0["r"]), :],
                        l0_sb[:, (b00 + q0) * 8: (b00 + q0 + qn) * 8],
                        qn * P,
                        qn * P,
                        P,
                        queue_num=next_q(),
                    )
                gb = lp.tile([P, LB * P], BF, tag="gb")
                for cl in lcalls1:
                    if not (b00 <= cl["blk0"] < b00 + nb0):
                        continue
                    r, nb = cl["r"], cl["nb"]
                    o = cl["blk0"] - b00
                    for q0 in range(0, nb, GMAX):
                        qn = min(GMAX, nb - q0)
                        nc.gpsimd.dma_gather(
                            gb[:, (o + q0) * P: (o + q0 + qn) * P].rearrange(
                                "p (g e) -> p g e", e=P
                            ),
                            o2_ag[r * cfg.rs: r * cfg.rs + rrows(r), :],
                            l1_sb[:, (cl["blk0"] + q0) * 8:
                                  (cl["blk0"] + q0 + qn) * 8],
                            qn * P,
                            qn * P,
                            P,
                            queue_num=next_q(),
                        )
                return ga, gb

            if phase >= 5:
                for cl0 in lcalls0:
                    ga, gb = lab_gather_region(cl0)
                    for ci in range(cl0["nb"]):
                        c = cl0["blk0"] + ci
                        sl = slice(ci * P, (ci + 1) * P)
                        prod = lp.tile([P, P], BF, tag="prod")
                        nc.vector.tensor_tensor(
                            out=prod[:], in0=ga[:, sl], in1=gb[:, sl],
                            op=OP.mult,
                        )
                        scr = lp.tile([P, P], BF, tag="scr")
                        nc.vector.tensor_tensor(
                            out=scr[:], in0=prod[:], in1=wv_sb[:], op=OP.mult
                        )
                        nc.vector.reduce_sum(
                            res_sb[:, c: c + 1], scr[:],
                            axis=mybir.AxisListType.X,
                        )
                nc.vector.tensor_scalar_add(
                    res_sb[:], res_sb[:], float(linb_sum)
                )
                nc.sync.dma_start(out=res_d[:, :], in_=res_sb[:])

    nc.finalize()
    return nc


# ------------------------------------------------------------------ driver


def make_in_maps(cfg, prep, W1, b1, W2, b2, lin_W, lin_b):
    consts = dict(
        ident=np.eye(P, dtype=np.float32).astype(ml_dtypes.bfloat16),
        w1=W1.astype(np.float32).astype(ml_dtypes.bfloat16),
        w2=W2.astype(np.float32).astype(ml_dtypes.bfloat16),
        b1bc=np.tile(b1.astype(np.float32)[None, :], (P, 1)),
        b2bc=np.tile(b2.astype(np.float32)[None, :], (P, 1)),
        wvbc=np.tile(
            lin_W.astype(np.float32).sum(axis=1)[None, :], (P, 1)
        ).astype(ml_dtypes.bfloat16),
    )
    in_maps = []
    for q in range(NC):
        m = dict(consts)
        m.update(
            xT=np.ascontiguousarray(
                prep["xT"][:, q * cfg.n_loc: (q + 1) * cfg.n_loc]
            ),
            wew=prep["wew"][q],
            gidx=prep["gidx"][q],
            l0idx=prep["l0idx"][q],
            l1idx=prep["l1idx"][q],
        )
        in_maps.append(m)
    return in_maps


def assemble_output(cfg, prep, results):
    out = np.zeros(cfg.n_labels, np.float32)
    for q in range(NC):
        r = np.asarray(results[q]["res"], np.float32)  # [128, LCP]
        flat = r.T.reshape(-1)  # slot j = c*128+p -> [c, p] row-major
        ids = prep["ids_slot"][q]
        m = ids >= 0
        out[ids[m]] = flat[m]
    return out


def run(cfg, x, edge_index, edge_weight, edge_label_index,
        W1, b1, W2, b2, lin_W, lin_b, trace=False, phase=99):
    global LAST_EXEC_NS, LAST_RESULTS
    prep = preprocess(cfg, np.asarray(x), np.asarray(edge_index),
                      np.asarray(edge_weight), np.asarray(edge_label_index))
    linb_sum = float(np.asarray(lin_b, np.float64).sum())
    nc = build_program(cfg, prep, linb_sum, phase=phase)
    in_maps = make_in_maps(cfg, prep, W1, b1, W2, b2, lin_W, lin_b)
    res = run_bass_kernel_spmd(
        nc, in_maps, list(range(NC)), trace=trace
    )
    LAST_EXEC_NS = res.exec_time_ns
    LAST_RESULTS = res
    return assemble_output(cfg, prep, res.results)


def kernel(x, edge_index, edge_weight, edge_label_index,
           W1, b1, W2, b2, lin_W, lin_b):
    trace = bool(os.environ.get("KERNEL_TRACE"))
    return run(FULL, x, edge_index, edge_weight, edge_label_index,
               W1, b1, W2, b2, lin_W, lin_b, trace=trace)


# revision 23
# speedup vs baseline: 1.5519x; 1.0146x over previous
"""GCN link-predictor kernel for 8 Trainium2 NeuronCores (Bass/Tile).

Strategy (SPMD, dst-sharded, v2):
  - Host: append self loops, assign each edge to the core owning its dst,
    group per 128-node dst tile, sort each tile's edges by src range
    (32768 rows = int16 dma_gather window), pad each (tile, range) to
    whole 128-edge chunks.  Ship the one-hot scatter matrices W_ew
    (W[e, dstl] = ew) pre-built in bf16, plus int16 local gather indices
    (16-partition wrapped) -- so the device does no one-hot construction.
  - Device per layer:  gather table rows with dma_gather (one call per
    supertile x src-range; ~1us SWDGE overhead amortized over thousands
    of rows), then chunk matmuls vs W_ew accumulate dst-tile partials in
    PSUM.  Layer1 table = dinv (.) (x @ W1) built from the core's own
    shard (lhsT = host-pre-transposed xT) and AllGathered; layer2 table
    = dinv (.) out1 produced directly at the layer1 evict (aggregate-
    then-GEMM reordering: (A X) W = A (X W)), so no full-table GEMM pass
    and no global dinv exchange -- all dinv folds are own-shard.
  - Labels: pairs sorted by (range(el0), range(el1)) into 16 groups so
    both sides gather via dma_gather; score = sum(a*b*wv) + sum(lin_b)
    with wv = lin_W @ 1.  Host un-permutes the result.
"""

import os
import sys

import numpy as np

for _p in ("/opt/trn_rl_repo",):
    if _p not in sys.path:
        sys.path.insert(0, _p)

import ml_dtypes  # noqa: E402

import concourse.bacc as bacc  # noqa: E402
import concourse.bass as bass  # noqa: E402
import concourse.mybir as mybir  # noqa: E402
from concourse.bass_utils import run_bass_kernel_spmd  # noqa: E402
from concourse.tile import TileContext  # noqa: E402

P = 128
NC = 8
RS = 32768  # dma_gather int16 window (rows)
SUP = 4    # dst tiles per supertile (gather-call granularity)
GMAX = 8   # max 128-row blocks per dma_gather call (SWDGE ring limit)
BF = mybir.dt.bfloat16
F32 = mybir.dt.float32
I16 = mybir.dt.int16

LAST_EXEC_NS = None
LAST_RESULTS = None


class Cfg:
    def __init__(self, n_nodes, n_labels, rs=RS):
        assert n_nodes % NC == 0
        self.n_nodes = n_nodes
        self.nodes_per_core = n_nodes // NC
        self.tiles_per_core = -(-self.nodes_per_core // P)
        self.n_loc = self.tiles_per_core * P
        self.n_pad = NC * self.n_loc
        self.n_labels = n_labels
        self.lab_per_core = -(-n_labels // NC)
        self.rs = rs
        self.nrg = -(-self.n_pad // rs)


FULL = Cfg(100000, 200000)


# ---------------------------------------------------------------- host prep


def _pad_ids(cfg, ids):
    q = np.minimum(ids // cfg.nodes_per_core, NC - 1)
    l = ids - q * cfg.nodes_per_core
    return q * cfg.n_loc + l


def _wrap16(flat_idx):
    # [n] -> [128, n//16]: idx j at [j%16, j//16], replicated to 128 parts
    n = len(flat_idx)
    assert n % 16 == 0
    a = np.zeros((16, n // 16), np.int16)
    a[np.arange(n) % 16, np.arange(n) // 16] = flat_idx
    return np.tile(a, (8, 1))


def preprocess(cfg, x, edge_index, edge_weight, edge_label_index):
    n = cfg.n_nodes
    T, NRG = cfg.tiles_per_core, cfg.nrg
    src = np.concatenate([edge_index[0], np.arange(n)]).astype(np.int64)
    dst = np.concatenate([edge_index[1], np.arange(n)]).astype(np.int64)
    ew = np.concatenate(
        [edge_weight.astype(np.float32), np.ones(n, np.float32)]
    )

    src_pad = _pad_ids(cfg, src)
    dst_pad = _pad_ids(cfg, dst)
    dq, dl = np.divmod(dst_pad, cfg.n_loc)
    dt_ = dl // P          # dst tile within core
    dloc = dl % P          # dst row within tile
    srange = src_pad // cfg.rs

    # chunk counts per (tile, range): max over cores for SPMD uniformity
    key = (dq * T + dt_) * NRG + srange
    counts = np.bincount(key, minlength=NC * T * NRG).reshape(NC, T, NRG)
    Kr = -(-counts.max(axis=0) // P)  # [T, NRG] blocks (may be 0)

    # supertile slab layout: for each supertile, range-major block order
    sups = []
    boff = np.zeros((T, NRG), np.int64)  # global block offset of (t, r)
    gcol = 0  # running gidx column offset
    nblk = 0
    for t0 in range(0, T, SUP):
        tiles = list(range(t0, min(t0 + SUP, T)))
        calls = []
        slab0 = nblk
        tile_chunks = {t: [] for t in tiles}
        for r in range(NRG):
            nb = int(sum(Kr[t, r] for t in tiles))
            if nb == 0:
                continue
            call_start = nblk
            for t in tiles:
                boff[t, r] = nblk
                tile_chunks[t].extend(range(nblk, nblk + int(Kr[t, r])))
                nblk += int(Kr[t, r])
            calls.append(dict(r=r, nb=nb, blk0=call_start, gcol=gcol))
            gcol += nb * 8  # nb*128/16 cols
        sups.append(dict(tiles=tiles, calls=calls, slab0=slab0,
                         nblk=nblk - slab0,
                         tile_chunks={t: [b - slab0 for b in tile_chunks[t]]
                                      for t in tiles}))
    C = nblk

    # slot assignment for every edge
    order = np.argsort(key, kind="stable")
    sk = key[order]
    starts = np.zeros(NC * T * NRG + 1, np.int64)
    starts[1:] = np.cumsum(counts.reshape(-1))
    pos = np.arange(len(order)) - starts[sk]
    e_core = order * 0 + dq[order]
    e_t, e_r = dt_[order], srange[order]
    blk = boff[e_t, e_r] + pos // P
    part = pos % P

    # per-slot dst-local index and edge weight, [P, C] each (slab order)
    slot_lin = blk * P + part
    dstl_a = np.zeros((NC, C * P), ml_dtypes.bfloat16)
    ewp_a = np.zeros((NC, C * P), ml_dtypes.bfloat16)
    dstl_a[e_core, slot_lin] = dloc[order].astype(ml_dtypes.bfloat16)
    ewp_a[e_core, slot_lin] = ew[order].astype(ml_dtypes.bfloat16)
    dstl_a = np.ascontiguousarray(
        dstl_a.reshape(NC, C, P).transpose(0, 2, 1))
    ewp_a = np.ascontiguousarray(
        ewp_a.reshape(NC, C, P).transpose(0, 2, 1))

    # gidx: per-call wrapped int16 local indices (idx j of a call lands at
    # [j%16, call_gcol + j//16], replicated 8x across partition groups)
    cid_blk0 = np.zeros((T, NRG), np.int64)
    cid_gcol = np.zeros((T, NRG), np.int64)
    for s in sups:
        for cl in s["calls"]:
            for t in s["tiles"]:
                cid_blk0[t, cl["r"]] = cl["blk0"]
                cid_gcol[t, cl["r"]] = cl["gcol"]
    loc_idx = (src_pad[order] - e_r * cfg.rs).astype(np.int64)
    j = (blk - cid_blk0[e_t, e_r]) * P + part
    gidx16 = np.zeros((NC, 16, gcol), np.int16)
    gidx16[e_core, j % 16, cid_gcol[e_t, e_r] + j // 16] = loc_idx
    gidx = np.tile(gidx16, (1, 8, 1))

    # ---- labels: sort by (range(el0), range(el1)) into NRG^2 groups ----
    el = edge_label_index.astype(np.int64)
    el0 = _pad_ids(cfg, el[0])
    el1 = _pad_ids(cfg, el[1])
    lpc = cfg.lab_per_core
    NG = NRG * NRG
    lab_ids = []      # per core: original label index per slot (-1 pad)
    lcounts = np.zeros((NC, NG), np.int64)
    per_core = []
    for q in range(NC):
        lo, hi = q * lpc, min((q + 1) * lpc, cfg.n_labels)
        ids = np.arange(lo, hi)
        g = (el0[ids] // cfg.rs) * NRG + (el1[ids] // cfg.rs)
        o = np.argsort(g, kind="stable")
        per_core.append((ids[o], g[o]))
        lcounts[q] = np.bincount(g, minlength=NG)
    Lg = -(-lcounts.max(axis=0) // P)  # [NG] blocks, max over cores
    LCP = int(Lg.sum())
    g_blk0 = np.zeros(NG + 1, np.int64)
    g_blk0[1:] = np.cumsum(Lg)

    l0flat = np.zeros((NC, LCP * P), np.int64)
    l1flat = np.zeros((NC, LCP * P), np.int64)
    ids_slot = -np.ones((NC, LCP * P), np.int64)
    for q in range(NC):
        ids_o, g_o = per_core[q]
        gstart = np.zeros(NG + 1, np.int64)
        gstart[1:] = np.cumsum(lcounts[q])
        posl = np.arange(len(ids_o)) - gstart[g_o]
        slot = g_blk0[g_o] * P + posl
        l0flat[q, slot] = el0[ids_o] - (g_o // NRG) * cfg.rs
        l1flat[q, slot] = el1[ids_o] - (g_o % NRG) * cfg.rs
        ids_slot[q, slot] = ids_o
    l0idx = np.stack([_wrap16(l0flat[q]) for q in range(NC)])
    l1idx = np.stack([_wrap16(l1flat[q]) for q in range(NC)])

    # label gather calls: el0 side = NRG calls (groups r0*NRG..r0*NRG+NRG-1),
    # el1 side = NG calls
    lcalls0 = []
    for r0 in range(NRG):
        nb = int(Lg[r0 * NRG: (r0 + 1) * NRG].sum())
        if nb:
            lcalls0.append(dict(r=r0, nb=nb, blk0=int(g_blk0[r0 * NRG])))
    lcalls1 = []
    for g in range(NG):
        nb = int(Lg[g])
        if nb:
            lcalls1.append(dict(r=g % NRG, nb=nb, blk0=int(g_blk0[g])))

    # node features: padded, transposed, own-shard sliced per core
    x_pad = np.zeros((cfg.n_pad, P), np.float32)
    x_pad[_pad_ids(cfg, np.arange(n))] = x
    xT = np.ascontiguousarray(x_pad.T).astype(ml_dtypes.bfloat16)

    bmax = max(su["nblk"] for su in sups)
    iota_rep = np.tile(np.arange(P, dtype=np.float32), (P, bmax)).astype(
        ml_dtypes.bfloat16)
    return dict(dstl=dstl_a, ewp=ewp_a, iota=iota_rep, gidx=gidx,
                l0idx=l0idx, l1idx=l1idx, xT=xT,
                sups=sups, C=C, LCP=LCP, lcalls0=lcalls0, lcalls1=lcalls1,
                ids_slot=ids_slot)


# ------------------------------------------------------------- bass program


def build_program(cfg, prep, linb_sum, phase=99):
    T, NRG = cfg.tiles_per_core, cfg.nrg
    NPAD, NLOC = cfg.n_pad, cfg.n_loc
    sups, C, LCP = prep["sups"], prep["C"], prep["LCP"]
    lcalls0, lcalls1 = prep["lcalls0"], prep["lcalls1"]
    GCOL = prep["gidx"].shape[2]
    BMAX = max(s["nblk"] for s in sups)
    rg = [list(range(NC))]

    def rrows(r):
        return min(cfg.rs, NPAD - r * cfg.rs)

    nc = bacc.Bacc(None, target_bir_lowering=False, debug=False,
                   num_swdge_queues=4)

    xT_d = nc.declare_dram_parameter("xT", [P, NLOC], BF, False)
    dstl_d = nc.declare_dram_parameter("dstl", [P, C], BF, False)
    ewp_d = nc.declare_dram_parameter("ewp", [P, C], BF, False)
    iota_d = nc.declare_dram_parameter("iota", [P, BMAX * P], BF, False)
    gidx_d = nc.declare_dram_parameter("gidx", [P, GCOL], I16, False)
    l0_d = nc.declare_dram_parameter("l0idx", [P, LCP * 8], I16, False)
    l1_d = nc.declare_dram_parameter("l1idx", [P, LCP * 8], I16, False)
    ident_d = nc.declare_dram_parameter("ident", [P, P], BF, False)
    w1_d = nc.declare_dram_parameter("w1", [P, P], BF, False)
    w2_d = nc.declare_dram_parameter("w2", [P, P], BF, False)
    b1_d = nc.declare_dram_parameter("b1bc", [P, P], F32, False)
    b2_d = nc.declare_dram_parameter("b2bc", [P, P], F32, False)
    wv_d = nc.declare_dram_parameter("wvbc", [P, P], BF, False)
    res_d = nc.declare_dram_parameter("res", [P, LCP], F32, True)

    t1_sh = nc.dram_tensor("t1_sh", [NLOC, P], BF)
    t1_ag = nc.dram_tensor("t1_ag", [NPAD, P], BF)
    t2_sh = nc.dram_tensor("t2_sh", [NLOC, P], BF)
    t2_ag = nc.dram_tensor("t2_ag", [NPAD, P], BF)
    o2_sh = nc.dram_tensor("o2_sh", [NLOC, P], BF)
    o2_ag = nc.dram_tensor("o2_ag", [NPAD, P], BF)

    AF = mybir.ActivationFunctionType
    OP = mybir.AluOpType
    qctr = [0]

    def next_q():
        qctr[0] = (qctr[0] + 1) % 4
        return qctr[0]

    with TileContext(nc) as tc:
        with (
            tc.tile_pool(name="const", bufs=1) as cp,
            tc.tile_pool(name="wslab", bufs=2) as wp,
            tc.tile_pool(name="gbuf", bufs=2) as gp,
            tc.tile_pool(name="idx", bufs=2) as ip,
            tc.tile_pool(name="xtile", bufs=3) as xp,
            tc.tile_pool(name="evict", bufs=4) as ep,
            tc.tile_pool(name="lab", bufs=1) as lp,
            tc.tile_pool(name="ps_deg", bufs=1, space="PSUM") as psd,
            tc.tile_pool(name="ps_agg", bufs=2, space="PSUM") as psa,
            tc.tile_pool(name="ps_gem", bufs=2, space="PSUM") as psg,
            tc.tile_pool(name="ps_tr", bufs=1, space="PSUM") as pst,
        ):
            # ---- persistent SBUF ----
            ident_sb = cp.tile([P, P], BF)
            nc.sync.dma_start(out=ident_sb[:], in_=ident_d[:, :])
            w1_sb = cp.tile([P, P], BF)
            nc.sync.dma_start(out=w1_sb[:], in_=w1_d[:, :])
            w2_sb = cp.tile([P, P], BF)
            nc.sync.dma_start(out=w2_sb[:], in_=w2_d[:, :])
            b1_sb = cp.tile([P, P], F32)
            nc.sync.dma_start(out=b1_sb[:], in_=b1_d[:, :])
            b2_sb = cp.tile([P, P], F32)
            nc.sync.dma_start(out=b2_sb[:], in_=b2_d[:, :])
            wv_sb = cp.tile([P, P], BF)
            nc.sync.dma_start(out=wv_sb[:], in_=wv_d[:, :])
            l0_sb = cp.tile([P, LCP * 8], I16)
            nc.sync.dma_start(out=l0_sb[:], in_=l0_d[:, :])
            l1_sb = cp.tile([P, LCP * 8], I16)
            nc.sync.dma_start(out=l1_sb[:], in_=l1_d[:, :])
            dstl_sb = cp.tile([P, C], BF)
            nc.sync.dma_start(out=dstl_sb[:], in_=dstl_d[:, :])
            ewp_sb = cp.tile([P, C], BF)
            nc.sync.dma_start(out=ewp_sb[:], in_=ewp_d[:, :])
            iota_sb = cp.tile([P, BMAX * P], BF)
            nc.sync.dma_start(out=iota_sb[:], in_=iota_d[:, :])
            iota3 = iota_sb[:].rearrange("p (g e) -> p g e", e=P)
            deg_sb = cp.tile([P, T], F32)
            rec_sb = cp.tile([P, T], F32)
            dinv_own = cp.tile([P, T], F32)
            res_sb = cp.tile([P, LCP], F32)

            def build_wslab(s, fold_ew):
                # one-hot W for the whole supertile slab in 1-2 DVE ops:
                # W[p, b, j] = (j == dstl[p, b]) [* ew[p, b]]
                w = wp.tile([P, BMAX * P], BF, tag="w")
                nb = s["nblk"]
                c0 = s["slab0"]
                w3 = w[:, : nb * P].rearrange("p (g e) -> p g e", e=P)
                nc.vector.tensor_tensor(
                    out=w3,
                    in0=iota3[:, :nb, :],
                    in1=dstl_sb[:, c0: c0 + nb].to_broadcast([P, nb, P]),
                    op=OP.is_equal,
                )
                if fold_ew:
                    nc.vector.tensor_tensor(
                        out=w3,
                        in0=w3,
                        in1=ewp_sb[:, c0: c0 + nb].to_broadcast([P, nb, P]),
                        op=OP.mult,
                    )
                return w

            # ---- deg pass (own tiles) ----
            for s in sups:
                w = build_wslab(s, False)
                for t in s["tiles"]:
                    blks = s["tile_chunks"][t]
                    pd = psd.tile([P, 1], F32)
                    for i, b in enumerate(blks):
                        c = s["slab0"] + b
                        nc.tensor.matmul(
                            out=pd[:],
                            lhsT=w[:, b * P: (b + 1) * P],
                            rhs=ewp_sb[:, c: c + 1],
                            start=(i == 0),
                            stop=(i == len(blks) - 1),
                        )
                    nc.scalar.activation(deg_sb[:, t: t + 1], pd[:], AF.Copy)
            nc.vector.tensor_scalar_max(deg_sb[:], deg_sb[:], 1.0)
            nc.vector.reciprocal(rec_sb[:], deg_sb[:])
            nc.scalar.activation(dinv_own[:], rec_sb[:], AF.Sqrt)

            # ---- layer-1 table: own shard of dinv*(x@W1), then AllGather
            for t in range(T):
                lhsT = xp.tile([P, P], BF, tag="lhsT")
                nc.sync.dma_start(
                    out=lhsT[:], in_=xT_d[:, t * P: (t + 1) * P]
                )
                pg = psg.tile([P, P], F32, tag="pg")
                nc.tensor.matmul(
                    out=pg[:], lhsT=lhsT[:], rhs=w1_sb[:],
                    start=True, stop=True,
                )
                hbf = xp.tile([P, P], BF, tag="hbf")
                nc.scalar.activation(
                    hbf[:], pg[:], AF.Copy, scale=dinv_own[:, t: t + 1]
                )
                nc.sync.dma_start(
                    out=t1_sh[t * P: (t + 1) * P, :], in_=hbf[:]
                )
            nc.gpsimd.collective_compute(
                "AllGather", OP.bypass, replica_groups=rg,
                ins=[t1_sh[:, :]], outs=[t1_ag[:, :]],
            )
            if phase == 1:
                pr = cp.tile([P, LCP], F32)
                nc.vector.tensor_copy(pr[:], dinv_own[:, :1].to_broadcast([P, LCP]))
                nc.sync.dma_start(out=res_d[:, :], in_=pr[:])

            # ---- aggregation supertile machinery ----
            def gather_sup(s, table):
                g = gp.tile([P, BMAX * P], BF, tag="g")
                for cl in s["calls"]:
                    r, nb = cl["r"], cl["nb"]
                    b0 = cl["blk0"] - s["slab0"]
                    it = ip.tile([P, BMAX * 8], I16, tag="gi")
                    nc.sync.dma_start(
                        out=it[:, : nb * 8],
                        in_=gidx_d[:, cl["gcol"]: cl["gcol"] + nb * 8],
                    )
                    for q0 in range(0, nb, GMAX):
                        qn = min(GMAX, nb - q0)
                        nc.gpsimd.dma_gather(
                            g[:, (b0 + q0) * P: (b0 + q0 + qn) * P].rearrange(
                                "p (g e) -> p g e", e=P
                            ),
                            table[r * cfg.rs: r * cfg.rs + rrows(r), :],
                            it[:, q0 * 8: (q0 + qn) * 8],
                            qn * P,
                            qn * P,
                            P,
                            queue_num=next_q(),
                        )
                return g

            # ---- layer 1: aggregate t1 -> out1, emit t2 = dinv*out1 ----
            def agg1():
                for s in sups:
                    w = build_wslab(s, True)
                    g = gather_sup(s, t1_ag)
                    for t in s["tiles"]:
                        blks = s["tile_chunks"][t]
                        pa = psa.tile([P, P], F32)
                        for i, b in enumerate(blks):
                            sl = slice(b * P, (b + 1) * P)
                            nc.tensor.matmul(
                                out=pa[:], lhsT=w[:, sl], rhs=g[:, sl],
                                start=(i == 0), stop=(i == len(blks) - 1),
                            )
                        t1 = ep.tile([P, P], F32, tag="t1")
                        nc.scalar.activation(
                            t1[:], pa[:], AF.Copy,
                            scale=dinv_own[:, t: t + 1],
                        )
                        nc.vector.tensor_tensor(
                            out=t1[:], in0=t1[:], in1=b1_sb[:], op=OP.add
                        )
                        o1 = ep.tile([P, P], F32, tag="o1")
                        nc.scalar.activation(o1[:], t1[:], AF.Relu)
                        t2b = ep.tile([P, P], BF, tag="t2b")
                        nc.scalar.activation(
                            t2b[:], o1[:], AF.Copy,
                            scale=dinv_own[:, t: t + 1],
                        )
                        nc.sync.dma_start(
                            out=t2_sh[t * P: (t + 1) * P, :], in_=t2b[:]
                        )

            # ---- layer 2: aggregate t2 (f-major psum), GEMM W2, evict ----
            def agg2():
                for s in sups:
                    w = build_wslab(s, True)
                    g = gather_sup(s, t2_ag)
                    for t in s["tiles"]:
                        blks = s["tile_chunks"][t]
                        pa = psa.tile([P, P], F32)
                        for i, b in enumerate(blks):
                            sl = slice(b * P, (b + 1) * P)
                            nc.tensor.matmul(
                                out=pa[:], lhsT=g[:, sl], rhs=w[:, sl],
                                start=(i == 0), stop=(i == len(blks) - 1),
                            )
                        uT = ep.tile([P, P], BF, tag="uT")
                        nc.scalar.activation(uT[:], pa[:], AF.Copy)
                        pb = psg.tile([P, P], F32, tag="pg")
                        nc.tensor.matmul(
                            out=pb[:], lhsT=w2_sb[:], rhs=uT[:],
                            start=True, stop=True,
                        )
                        vT = ep.tile([P, P], BF, tag="vT")
                        nc.scalar.activation(vT[:], pb[:], AF.Copy)
                        pt = pst.tile([P, P], BF)
                        nc.tensor.transpose(
                            out=pt[:], in_=vT[:], identity=ident_sb[:]
                        )
                        t1 = ep.tile([P, P], F32, tag="t1")
                        nc.scalar.activation(
                            t1[:], pt[:], AF.Copy,
                            scale=dinv_own[:, t: t + 1],
                        )
                        nc.vector.tensor_tensor(
                            out=t1[:], in0=t1[:], in1=b2_sb[:], op=OP.add
                        )
                        o2 = ep.tile([P, P], BF, tag="o2")
                        nc.scalar.activation(o2[:], t1[:], AF.Relu)
                        nc.sync.dma_start(
                            out=o2_sh[t * P: (t + 1) * P, :], in_=o2[:]
                        )

            def probe(src):
                pr = cp.tile([P, P], BF)
                nc.sync.dma_start(out=pr[:], in_=src)
                prf = cp.tile([P, P], F32)
                nc.vector.tensor_copy(prf[:], pr[:])
                nc.sync.dma_start(out=res_d[:, : min(LCP, P)],
                                  in_=prf[:, : min(LCP, P)])

            if phase >= 2:
                agg1()
            if phase == 2:
                probe(t2_sh[0:P, :])
            if phase >= 3:
                nc.gpsimd.collective_compute(
                    "AllGather", OP.bypass, replica_groups=rg,
                    ins=[t2_sh[:, :]], outs=[t2_ag[:, :]],
                )
            if phase == 3:
                probe(t2_ag[0:P, :])
            if phase >= 4:
                agg2()
                nc.gpsimd.collective_compute(
                    "AllGather", OP.bypass, replica_groups=rg,
                    ins=[o2_sh[:, :]], outs=[o2_ag[:, :]],
                )
            if phase == 4:
                probe(o2_ag[0:P, :])

            # ---- label pass: process one el0-range region at a time ----
            LB = max((cl["nb"] for cl in lcalls0), default=1)

            def lab_gather_region(cl0):
                b00, nb0 = cl0["blk0"], cl0["nb"]
                ga = lp.tile([P, LB * P], BF, tag="ga")
                for q0 in range(0, nb0, GMAX):
                    qn = min(GMAX, nb0 - q0)
                    nc.gpsimd.dma_gather(
                        ga[:, q0 * P: (q0 + qn) * P].rearrange(
                            "p (g e) -> p g e", e=P
                        ),
                        o2_ag[cl0["r"] * cfg.rs: cl0["r"] * cfg.rs
                              + rrows(cl0["r"]), :],
                        l0_sb[:, (b00 + q0) * 8: (b00 + q0 + qn) * 8],
                        qn * P,
                        qn * P,
                        P,
                        queue_num=next_q(),
                    )
                gb = lp.tile([P, LB * P], BF, tag="gb")
                for cl in lcalls1:
                    if not (b00 <= cl["blk0"] < b00 + nb0):
                        continue
                    r, nb = cl["r"], cl["nb"]
                    o = cl["blk0"] - b00
                    for q0 in range(0, nb, GMAX):
                        qn = min(GMAX, nb - q0)
                        nc.gpsimd.dma_gather(
                            gb[:, (o + q0) * P: (o + q0 + qn) * P].rearrange(
                                "p (g e) -> p g e", e=P
                            ),
                            o2_ag[r * cfg.rs: r * cfg.rs + rrows(r), :],
                            l1_sb[:, (cl["blk0"] + q0) * 8:
                                  (cl["blk0"] + q0 + qn) * 8],
                            qn * P,
                            qn * P,
                            P,
                            queue_num=next_q(),
                        )
                return ga, gb

            if phase >= 5:
                for cl0 in lcalls0:
                    ga, gb = lab_gather_region(cl0)
                    for ci in range(cl0["nb"]):
                        c = cl0["blk0"] + ci
                        sl = slice(ci * P, (ci + 1) * P)
                        prod = lp.tile([P, P], BF, tag="prod")
                        nc.vector.tensor_tensor(
                            out=prod[:], in0=ga[:, sl], in1=gb[:, sl],
                            op=OP.mult,
                        )
                        scr = lp.tile([P, P], BF, tag="scr")
                        nc.vector.tensor_tensor(
                            out=scr[:], in0=prod[:], in1=wv_sb[:], op=OP.mult
                        )
                        nc.vector.reduce_sum(
                            res_sb[:, c: c + 1], scr[:],
                            axis=mybir.AxisListType.X,
                        )
                nc.vector.tensor_scalar_add(
                    res_sb[:], res_sb[:], float(linb_sum)
                )
                nc.sync.dma_start(out=res_d[:, :], in_=res_sb[:])

    nc.finalize()
    return nc


# ------------------------------------------------------------------ driver


def make_in_maps(cfg, prep, W1, b1, W2, b2, lin_W, lin_b):
    consts = dict(
        ident=np.eye(P, dtype=np.float32).astype(ml_dtypes.bfloat16),
        w1=W1.astype(np.float32).astype(ml_dtypes.bfloat16),
        w2=W2.astype(np.float32).astype(ml_dtypes.bfloat16),
        b1bc=np.tile(b1.astype(np.float32)[None, :], (P, 1)),
        b2bc=np.tile(b2.astype(np.float32)[None, :], (P, 1)),
        wvbc=np.tile(
            lin_W.astype(np.float32).sum(axis=1)[None, :], (P, 1)
        ).astype(ml_dtypes.bfloat16),
    )
    in_maps = []
    for q in range(NC):
        m = dict(consts)
        m.update(
            xT=np.ascontiguousarray(
                prep["xT"][:, q * cfg.n_loc: (q + 1) * cfg.n_loc]
            ),
            dstl=prep["dstl"][q],
            ewp=prep["ewp"][q],
            iota=prep["iota"],
            gidx=prep["gidx"][q],
            l0idx=prep["l0idx"][q],
            l1idx=prep["l1idx"][q],
        )
        in_maps.append(m)
    return in_maps


def assemble_output(cfg, prep, results):
    out = np.zeros(cfg.n_labels, np.float32)
    for q in range(NC):
        r = np.asarray(results[q]["res"], np.float32)  # [128, LCP]
        flat = r.T.reshape(-1)  # slot j = c*128+p -> [c, p] row-major
        ids = prep["ids_slot"][q]
        m = ids >= 0
        out[ids[m]] = flat[m]
    return out


def run(cfg, x, edge_index, edge_weight, edge_label_index,
        W1, b1, W2, b2, lin_W, lin_b, trace=False, phase=99):
    global LAST_EXEC_NS, LAST_RESULTS
    prep = preprocess(cfg, np.asarray(x), np.asarray(edge_index),
                      np.asarray(edge_weight), np.asarray(edge_label_index))
    linb_sum = float(np.asarray(lin_b, np.float64).sum())
    nc = build_program(cfg, prep, linb_sum, phase=phase)
    in_maps = make_in_maps(cfg, prep, W1, b1, W2, b2, lin_W, lin_b)
    res = run_bass_kernel_spmd(
        nc, in_maps, list(range(NC)), trace=trace
    )
    LAST_EXEC_NS = res.exec_time_ns
    LAST_RESULTS = res
    return assemble_output(cfg, prep, res.results)


def kernel(x, edge_index, edge_weight, edge_label_index,
           W1, b1, W2, b2, lin_W, lin_b):
    trace = bool(os.environ.get("KERNEL_TRACE"))
    return run(FULL, x, edge_index, edge_weight, edge_label_index,
               W1, b1, W2, b2, lin_W, lin_b, trace=trace)


# revision 27
# speedup vs baseline: 1.5577x; 1.0037x over previous
"""GCN link-predictor kernel for 8 Trainium2 NeuronCores (Bass/Tile).

Strategy (SPMD, dst-sharded, v2):
  - Host: append self loops, assign each edge to the core owning its dst,
    group per 128-node dst tile, sort each tile's edges by src range
    (32768 rows = int16 dma_gather window), pad each (tile, range) to
    whole 128-edge chunks.  Ship the one-hot scatter matrices W_ew
    (W[e, dstl] = ew) pre-built in bf16, plus int16 local gather indices
    (16-partition wrapped) -- so the device does no one-hot construction.
  - Device per layer:  gather table rows with dma_gather (one call per
    supertile x src-range; ~1us SWDGE overhead amortized over thousands
    of rows), then chunk matmuls vs W_ew accumulate dst-tile partials in
    PSUM.  Layer1 table = dinv (.) (x @ W1) built from the core's own
    shard (lhsT = host-pre-transposed xT) and AllGathered; layer2 table
    = dinv (.) out1 produced directly at the layer1 evict (aggregate-
    then-GEMM reordering: (A X) W = A (X W)), so no full-table GEMM pass
    and no global dinv exchange -- all dinv folds are own-shard.
  - Labels: pairs sorted by (range(el0), range(el1)) into 16 groups so
    both sides gather via dma_gather; score = sum(a*b*wv) + sum(lin_b)
    with wv = lin_W @ 1.  Host un-permutes the result.
"""

import os
import sys

import numpy as np

for _p in ("/opt/trn_rl_repo",):
    if _p not in sys.path:
        sys.path.insert(0, _p)

import ml_dtypes  # noqa: E402

import concourse.bacc as bacc  # noqa: E402
import concourse.bass as bass  # noqa: E402
import concourse.mybir as mybir  # noqa: E402
from concourse.bass_utils import run_bass_kernel_spmd  # noqa: E402
from concourse.tile import TileContext  # noqa: E402

P = 128
NC = 8
RS = 32768  # dma_gather int16 window (rows)
SUP = 4    # dst tiles per supertile (gather-call granularity)
GMAX = 8   # max 128-row blocks per dma_gather call (SWDGE ring limit)
BF = mybir.dt.bfloat16
F32 = mybir.dt.float32
I16 = mybir.dt.int16

LAST_EXEC_NS = None
LAST_RESULTS = None


class Cfg:
    def __init__(self, n_nodes, n_labels, rs=RS):
        assert n_nodes % NC == 0
        self.n_nodes = n_nodes
        self.nodes_per_core = n_nodes // NC
        self.tiles_per_core = -(-self.nodes_per_core // P)
        self.n_loc = self.tiles_per_core * P
        self.n_pad = NC * self.n_loc
        self.n_labels = n_labels
        self.lab_per_core = -(-n_labels // NC)
        self.rs = rs
        self.nrg = -(-self.n_pad // rs)


FULL = Cfg(100000, 200000)


# ---------------------------------------------------------------- host prep


def _pad_ids(cfg, ids):
    q = np.minimum(ids // cfg.nodes_per_core, NC - 1)
    l = ids - q * cfg.nodes_per_core
    return q * cfg.n_loc + l


def _wrap16(flat_idx):
    # [n] -> [128, n//16]: idx j at [j%16, j//16], replicated to 128 parts
    n = len(flat_idx)
    assert n % 16 == 0
    a = np.zeros((16, n // 16), np.int16)
    a[np.arange(n) % 16, np.arange(n) // 16] = flat_idx
    return np.tile(a, (8, 1))


def preprocess(cfg, x, edge_index, edge_weight, edge_label_index):
    n = cfg.n_nodes
    T, NRG = cfg.tiles_per_core, cfg.nrg
    src = np.concatenate([edge_index[0], np.arange(n)]).astype(np.int64)
    dst = np.concatenate([edge_index[1], np.arange(n)]).astype(np.int64)
    ew = np.concatenate(
        [edge_weight.astype(np.float32), np.ones(n, np.float32)]
    )

    src_pad = _pad_ids(cfg, src)
    dst_pad = _pad_ids(cfg, dst)
    dq, dl = np.divmod(dst_pad, cfg.n_loc)
    dt_ = dl // P          # dst tile within core
    dloc = dl % P          # dst row within tile
    srange = src_pad // cfg.rs

    # chunk counts per (tile, range): max over cores for SPMD uniformity
    key = (dq * T + dt_) * NRG + srange
    counts = np.bincount(key, minlength=NC * T * NRG).reshape(NC, T, NRG)
    Kr = -(-counts.max(axis=0) // P)  # [T, NRG] blocks (may be 0)

    # supertile slab layout: for each supertile, range-major block order
    sups = []
    boff = np.zeros((T, NRG), np.int64)  # global block offset of (t, r)
    gcol = 0  # running gidx column offset
    nblk = 0
    for t0 in range(0, T, SUP):
        tiles = list(range(t0, min(t0 + SUP, T)))
        calls = []
        slab0 = nblk
        tile_chunks = {t: [] for t in tiles}
        for r in range(NRG):
            nb = int(sum(Kr[t, r] for t in tiles))
            if nb == 0:
                continue
            call_start = nblk
            for t in tiles:
                boff[t, r] = nblk
                tile_chunks[t].extend(range(nblk, nblk + int(Kr[t, r])))
                nblk += int(Kr[t, r])
            calls.append(dict(r=r, nb=nb, blk0=call_start, gcol=gcol))
            gcol += nb * 8  # nb*128/16 cols
        sups.append(dict(tiles=tiles, calls=calls, slab0=slab0,
                         nblk=nblk - slab0,
                         tile_chunks={t: [b - slab0 for b in tile_chunks[t]]
                                      for t in tiles}))
    C = nblk

    # slot assignment for every edge
    order = np.argsort(key, kind="stable")
    sk = key[order]
    starts = np.zeros(NC * T * NRG + 1, np.int64)
    starts[1:] = np.cumsum(counts.reshape(-1))
    pos = np.arange(len(order)) - starts[sk]
    e_core = order * 0 + dq[order]
    e_t, e_r = dt_[order], srange[order]
    blk = boff[e_t, e_r] + pos // P
    part = pos % P

    # per-slot dst-local index and edge weight, [P, C] each (slab order)
    slot_lin = blk * P + part
    dstl_a = np.zeros((NC, C * P), ml_dtypes.bfloat16)
    ewp_a = np.zeros((NC, C * P), ml_dtypes.bfloat16)
    dstl_a[e_core, slot_lin] = dloc[order].astype(ml_dtypes.bfloat16)
    ewp_a[e_core, slot_lin] = ew[order].astype(ml_dtypes.bfloat16)
    dstl_a = np.ascontiguousarray(
        dstl_a.reshape(NC, C, P).transpose(0, 2, 1))
    ewp_a = np.ascontiguousarray(
        ewp_a.reshape(NC, C, P).transpose(0, 2, 1))

    # gidx: per-call wrapped int16 local indices (idx j of a call lands at
    # [j%16, call_gcol + j//16], replicated 8x across partition groups)
    cid_blk0 = np.zeros((T, NRG), np.int64)
    cid_gcol = np.zeros((T, NRG), np.int64)
    for s in sups:
        for cl in s["calls"]:
            for t in s["tiles"]:
                cid_blk0[t, cl["r"]] = cl["blk0"]
                cid_gcol[t, cl["r"]] = cl["gcol"]
    loc_idx = (src_pad[order] - e_r * cfg.rs).astype(np.int64)
    j = (blk - cid_blk0[e_t, e_r]) * P + part
    gidx16 = np.zeros((NC, 16, gcol), np.int16)
    gidx16[e_core, j % 16, cid_gcol[e_t, e_r] + j // 16] = loc_idx
    gidx = np.tile(gidx16, (1, 8, 1))

    # ---- labels: sort by (range(el0), range(el1)) into NRG^2 groups ----
    el = edge_label_index.astype(np.int64)
    el0 = _pad_ids(cfg, el[0])
    el1 = _pad_ids(cfg, el[1])
    lpc = cfg.lab_per_core
    NG = NRG * NRG
    lab_ids = []      # per core: original label index per slot (-1 pad)
    lcounts = np.zeros((NC, NG), np.int64)
    per_core = []
    for q in range(NC):
        lo, hi = q * lpc, min((q + 1) * lpc, cfg.n_labels)
        ids = np.arange(lo, hi)
        g = (el0[ids] // cfg.rs) * NRG + (el1[ids] // cfg.rs)
        o = np.argsort(g, kind="stable")
        per_core.append((ids[o], g[o]))
        lcounts[q] = np.bincount(g, minlength=NG)
    Lg = -(-lcounts.max(axis=0) // P)  # [NG] blocks, max over cores
    LCP = int(Lg.sum())
    g_blk0 = np.zeros(NG + 1, np.int64)
    g_blk0[1:] = np.cumsum(Lg)

    l0flat = np.zeros((NC, LCP * P), np.int64)
    l1flat = np.zeros((NC, LCP * P), np.int64)
    ids_slot = -np.ones((NC, LCP * P), np.int64)
    for q in range(NC):
        ids_o, g_o = per_core[q]
        gstart = np.zeros(NG + 1, np.int64)
        gstart[1:] = np.cumsum(lcounts[q])
        posl = np.arange(len(ids_o)) - gstart[g_o]
        slot = g_blk0[g_o] * P + posl
        l0flat[q, slot] = el0[ids_o] - (g_o // NRG) * cfg.rs
        l1flat[q, slot] = el1[ids_o] - (g_o % NRG) * cfg.rs
        ids_slot[q, slot] = ids_o
    l0idx = np.stack([_wrap16(l0flat[q]) for q in range(NC)])
    l1idx = np.stack([_wrap16(l1flat[q]) for q in range(NC)])

    # label gather calls: el0 side = NRG calls (groups r0*NRG..r0*NRG+NRG-1),
    # el1 side = NG calls
    lcalls0 = []
    for r0 in range(NRG):
        nb = int(Lg[r0 * NRG: (r0 + 1) * NRG].sum())
        if nb:
            lcalls0.append(dict(r=r0, nb=nb, blk0=int(g_blk0[r0 * NRG])))
    lcalls1 = []
    for g in range(NG):
        nb = int(Lg[g])
        if nb:
            lcalls1.append(dict(r=g % NRG, nb=nb, blk0=int(g_blk0[g])))

    # node features: padded, transposed, own-shard sliced per core
    x_pad = np.zeros((cfg.n_pad, P), np.float32)
    x_pad[_pad_ids(cfg, np.arange(n))] = x
    xT = np.ascontiguousarray(x_pad.T).astype(ml_dtypes.bfloat16)

    bmax = max(su["nblk"] for su in sups)
    iota_rep = np.tile(np.arange(P, dtype=np.float32), (P, bmax)).astype(
        ml_dtypes.bfloat16)
    return dict(dstl=dstl_a, ewp=ewp_a, iota=iota_rep, gidx=gidx,
                l0idx=l0idx, l1idx=l1idx, xT=xT,
                sups=sups, C=C, LCP=LCP, lcalls0=lcalls0, lcalls1=lcalls1,
                ids_slot=ids_slot)


# ------------------------------------------------------------- bass program


def build_program(cfg, prep, linb_sum, phase=99):
    T, NRG = cfg.tiles_per_core, cfg.nrg
    NPAD, NLOC = cfg.n_pad, cfg.n_loc
    sups, C, LCP = prep["sups"], prep["C"], prep["LCP"]
    lcalls0, lcalls1 = prep["lcalls0"], prep["lcalls1"]
    GCOL = prep["gidx"].shape[2]
    BMAX = max(s["nblk"] for s in sups)
    rg = [list(range(NC))]

    def rrows(r):
        return min(cfg.rs, NPAD - r * cfg.rs)

    nc = bacc.Bacc(None, target_bir_lowering=False, debug=False,
                   num_swdge_queues=4)

    xT_d = nc.declare_dram_parameter("xT", [P, NLOC], BF, False)
    dstl_d = nc.declare_dram_parameter("dstl", [P, C], BF, False)
    ewp_d = nc.declare_dram_parameter("ewp", [P, C], BF, False)
    iota_d = nc.declare_dram_parameter("iota", [P, BMAX * P], BF, False)
    gidx_d = nc.declare_dram_parameter("gidx", [P, GCOL], I16, False)
    l0_d = nc.declare_dram_parameter("l0idx", [P, LCP * 8], I16, False)
    l1_d = nc.declare_dram_parameter("l1idx", [P, LCP * 8], I16, False)
    ident_d = nc.declare_dram_parameter("ident", [P, P], BF, False)
    w1_d = nc.declare_dram_parameter("w1", [P, P], BF, False)
    w2_d = nc.declare_dram_parameter("w2", [P, P], BF, False)
    b1_d = nc.declare_dram_parameter("b1bc", [P, P], F32, False)
    b2_d = nc.declare_dram_parameter("b2bc", [P, P], F32, False)
    wv_d = nc.declare_dram_parameter("wvbc", [P, P], BF, False)
    res_d = nc.declare_dram_parameter("res", [P, LCP], F32, True)

    t1_sh = nc.dram_tensor("t1_sh", [NLOC, P], BF)
    t1_ag = nc.dram_tensor("t1_ag", [NPAD, P], BF)
    t2_sh = nc.dram_tensor("t2_sh", [NLOC, P], BF)
    t2_ag = nc.dram_tensor("t2_ag", [NPAD, P], BF)
    o2_sh = nc.dram_tensor("o2_sh", [NLOC, P], BF)
    o2_ag = nc.dram_tensor("o2_ag", [NPAD, P], BF)

    AF = mybir.ActivationFunctionType
    OP = mybir.AluOpType
    qctr = [0]

    def next_q():
        qctr[0] = (qctr[0] + 1) % 4
        return qctr[0]

    with TileContext(nc) as tc:
        with (
            tc.tile_pool(name="const", bufs=1) as cp,
            tc.tile_pool(name="wslab", bufs=2) as wp,
            tc.tile_pool(name="gbuf", bufs=2) as gp,
            tc.tile_pool(name="idx", bufs=8) as ip,
            tc.tile_pool(name="xtile", bufs=3) as xp,
            tc.tile_pool(name="evict", bufs=4) as ep,
            tc.tile_pool(name="lab", bufs=1) as lp,
            tc.tile_pool(name="ps_deg", bufs=1, space="PSUM") as psd,
            tc.tile_pool(name="ps_agg", bufs=2, space="PSUM") as psa,
            tc.tile_pool(name="ps_gem", bufs=2, space="PSUM") as psg,
            tc.tile_pool(name="ps_tr", bufs=1, space="PSUM") as pst,
        ):
            # ---- persistent SBUF ----
            ident_sb = cp.tile([P, P], BF)
            nc.sync.dma_start(out=ident_sb[:], in_=ident_d[:, :])
            w1_sb = cp.tile([P, P], BF)
            nc.sync.dma_start(out=w1_sb[:], in_=w1_d[:, :])
            w2_sb = cp.tile([P, P], BF)
            nc.sync.dma_start(out=w2_sb[:], in_=w2_d[:, :])
            b1_sb = cp.tile([P, P], F32)
            nc.sync.dma_start(out=b1_sb[:], in_=b1_d[:, :])
            b2_sb = cp.tile([P, P], F32)
            nc.sync.dma_start(out=b2_sb[:], in_=b2_d[:, :])
            wv_sb = cp.tile([P, P], BF)
            nc.sync.dma_start(out=wv_sb[:], in_=wv_d[:, :])
            l0_sb = cp.tile([P, LCP * 8], I16)
            nc.sync.dma_start(out=l0_sb[:], in_=l0_d[:, :])
            l1_sb = cp.tile([P, LCP * 8], I16)
            nc.sync.dma_start(out=l1_sb[:], in_=l1_d[:, :])
            dstl_sb = cp.tile([P, C], BF)
            nc.sync.dma_start(out=dstl_sb[:], in_=dstl_d[:, :])
            ewp_sb = cp.tile([P, C], BF)
            nc.sync.dma_start(out=ewp_sb[:], in_=ewp_d[:, :])
            iota_sb = cp.tile([P, BMAX * P], BF)
            nc.sync.dma_start(out=iota_sb[:], in_=iota_d[:, :])
            iota3 = iota_sb[:].rearrange("p (g e) -> p g e", e=P)
            deg_sb = cp.tile([P, T], F32)
            rec_sb = cp.tile([P, T], F32)
            dinv_own = cp.tile([P, T], F32)
            res_sb = cp.tile([P, LCP], F32)

            def build_wslab(s, fold_ew):
                # one-hot W for the whole supertile slab in 1-2 DVE ops:
                # W[p, b, j] = (j == dstl[p, b]) [* ew[p, b]]
                w = wp.tile([P, BMAX * P], BF, tag="w")
                nb = s["nblk"]
                c0 = s["slab0"]
                w3 = w[:, : nb * P].rearrange("p (g e) -> p g e", e=P)
                nc.vector.tensor_tensor(
                    out=w3,
                    in0=iota3[:, :nb, :],
                    in1=dstl_sb[:, c0: c0 + nb].to_broadcast([P, nb, P]),
                    op=OP.is_equal,
                )
                if fold_ew:
                    nc.vector.tensor_tensor(
                        out=w3,
                        in0=w3,
                        in1=ewp_sb[:, c0: c0 + nb].to_broadcast([P, nb, P]),
                        op=OP.mult,
                    )
                return w

            # ---- deg pass (own tiles) ----
            for s in sups:
                w = build_wslab(s, False)
                for t in s["tiles"]:
                    blks = s["tile_chunks"][t]
                    pd = psd.tile([P, 1], F32)
                    for i, b in enumerate(blks):
                        c = s["slab0"] + b
                        nc.tensor.matmul(
                            out=pd[:],
                            lhsT=w[:, b * P: (b + 1) * P],
                            rhs=ewp_sb[:, c: c + 1],
                            start=(i == 0),
                            stop=(i == len(blks) - 1),
                        )
                    nc.scalar.activation(deg_sb[:, t: t + 1], pd[:], AF.Copy)
            nc.vector.tensor_scalar_max(deg_sb[:], deg_sb[:], 1.0)
            nc.vector.reciprocal(rec_sb[:], deg_sb[:])
            nc.scalar.activation(dinv_own[:], rec_sb[:], AF.Sqrt)

            # ---- layer-1 table: own shard of dinv*(x@W1), then AllGather
            for t in range(T):
                lhsT = xp.tile([P, P], BF, tag="lhsT")
                nc.sync.dma_start(
                    out=lhsT[:], in_=xT_d[:, t * P: (t + 1) * P]
                )
                pg = psg.tile([P, P], F32, tag="pg")
                nc.tensor.matmul(
                    out=pg[:], lhsT=lhsT[:], rhs=w1_sb[:],
                    start=True, stop=True,
                )
                hbf = xp.tile([P, P], BF, tag="hbf")
                nc.scalar.activation(
                    hbf[:], pg[:], AF.Copy, scale=dinv_own[:, t: t + 1]
                )
                nc.sync.dma_start(
                    out=t1_sh[t * P: (t + 1) * P, :], in_=hbf[:]
                )
            nc.gpsimd.collective_compute(
                "AllGather", OP.bypass, replica_groups=rg,
                ins=[t1_sh[:, :]], outs=[t1_ag[:, :]],
            )
            if phase == 1:
                pr = cp.tile([P, LCP], F32)
                nc.vector.tensor_copy(pr[:], dinv_own[:, :1].to_broadcast([P, LCP]))
                nc.sync.dma_start(out=res_d[:, :], in_=pr[:])

            # ---- aggregation supertile machinery ----
            def gather_sup(s, table):
                g = gp.tile([P, BMAX * P], BF, tag="g")
                for cl in s["calls"]:
                    r, nb = cl["r"], cl["nb"]
                    b0 = cl["blk0"] - s["slab0"]
                    it = ip.tile([P, BMAX * 8], I16, tag="gi")
                    nc.sync.dma_start(
                        out=it[:, : nb * 8],
                        in_=gidx_d[:, cl["gcol"]: cl["gcol"] + nb * 8],
                    )
                    for q0 in range(0, nb, GMAX):
                        qn = min(GMAX, nb - q0)
                        nc.gpsimd.dma_gather(
                            g[:, (b0 + q0) * P: (b0 + q0 + qn) * P].rearrange(
                                "p (g e) -> p g e", e=P
                            ),
                            table[r * cfg.rs: r * cfg.rs + rrows(r), :],
                            it[:, q0 * 8: (q0 + qn) * 8],
                            qn * P,
                            qn * P,
                            P,
                            queue_num=next_q(),
                        )
                return g

            # ---- layer 1: aggregate t1 -> out1, emit t2 = dinv*out1 ----
            def agg1():
                for s in sups:
                    w = build_wslab(s, True)
                    g = gather_sup(s, t1_ag)
                    for t in s["tiles"]:
                        blks = s["tile_chunks"][t]
                        pa = psa.tile([P, P], F32)
                        for i, b in enumerate(blks):
                            sl = slice(b * P, (b + 1) * P)
                            nc.tensor.matmul(
                                out=pa[:], lhsT=w[:, sl], rhs=g[:, sl],
                                start=(i == 0), stop=(i == len(blks) - 1),
                            )
                        t1 = ep.tile([P, P], F32, tag="t1")
                        nc.scalar.activation(
                            t1[:], pa[:], AF.Copy,
                            scale=dinv_own[:, t: t + 1],
                        )
                        nc.vector.tensor_tensor(
                            out=t1[:], in0=t1[:], in1=b1_sb[:], op=OP.add
                        )
                        o1 = ep.tile([P, P], F32, tag="o1")
                        nc.scalar.activation(o1[:], t1[:], AF.Relu)
                        t2b = ep.tile([P, P], BF, tag="t2b")
                        nc.scalar.activation(
                            t2b[:], o1[:], AF.Copy,
                            scale=dinv_own[:, t: t + 1],
                        )
                        nc.sync.dma_start(
                            out=t2_sh[t * P: (t + 1) * P, :], in_=t2b[:]
                        )

            # ---- layer 2: aggregate t2 (f-major psum), GEMM W2, evict ----
            def agg2():
                for s in sups:
                    w = build_wslab(s, True)
                    g = gather_sup(s, t2_ag)
                    for t in s["tiles"]:
                        blks = s["tile_chunks"][t]
                        pa = psa.tile([P, P], F32)
                        for i, b in enumerate(blks):
                            sl = slice(b * P, (b + 1) * P)
                            nc.tensor.matmul(
                                out=pa[:], lhsT=g[:, sl], rhs=w[:, sl],
                                start=(i == 0), stop=(i == len(blks) - 1),
                            )
                        uT = ep.tile([P, P], BF, tag="uT")
                        nc.scalar.activation(uT[:], pa[:], AF.Copy)
                        pb = psg.tile([P, P], F32, tag="pg")
                        nc.tensor.matmul(
                            out=pb[:], lhsT=w2_sb[:], rhs=uT[:],
                            start=True, stop=True,
                        )
                        vT = ep.tile([P, P], BF, tag="vT")
                        nc.scalar.activation(vT[:], pb[:], AF.Copy)
                        pt = pst.tile([P, P], BF)
                        nc.tensor.transpose(
                            out=pt[:], in_=vT[:], identity=ident_sb[:]
                        )
                        t1 = ep.tile([P, P], F32, tag="t1")
                        nc.scalar.activation(
                            t1[:], pt[:], AF.Copy,
                            scale=dinv_own[:, t: t + 1],
                        )
                        nc.vector.tensor_tensor(
                            out=t1[:], in0=t1[:], in1=b2_sb[:], op=OP.add
                        )
                        o2 = ep.tile([P, P], BF, tag="o2")
                        nc.scalar.activation(o2[:], t1[:], AF.Relu)
                        nc.sync.dma_start(
                            out=o2_sh[t * P: (t + 1) * P, :], in_=o2[:]
                        )

            def probe(src):
                pr = cp.tile([P, P], BF)
                nc.sync.dma_start(out=pr[:], in_=src)
                prf = cp.tile([P, P], F32)
                nc.vector.tensor_copy(prf[:], pr[:])
                nc.sync.dma_start(out=res_d[:, : min(LCP, P)],
                                  in_=prf[:, : min(LCP, P)])

            if phase >= 2:
                agg1()
            if phase == 2:
                probe(t2_sh[0:P, :])
            if phase >= 3:
                nc.gpsimd.collective_compute(
                    "AllGather", OP.bypass, replica_groups=rg,
                    ins=[t2_sh[:, :]], outs=[t2_ag[:, :]],
                )
            if phase == 3:
                probe(t2_ag[0:P, :])
            if phase >= 4:
                agg2()
                nc.gpsimd.collective_compute(
                    "AllGather", OP.bypass, replica_groups=rg,
                    ins=[o2_sh[:, :]], outs=[o2_ag[:, :]],
                )
            if phase == 4:
                probe(o2_ag[0:P, :])

            # ---- label pass: process one el0-range region at a time ----
            LB = max((cl["nb"] for cl in lcalls0), default=1)

            def lab_gather_region(cl0):
                b00, nb0 = cl0["blk0"], cl0["nb"]
                ga = lp.tile([P, LB * P], BF, tag="ga")
                for q0 in range(0, nb0, GMAX):
                    qn = min(GMAX, nb0 - q0)
                    nc.gpsimd.dma_gather(
                        ga[:, q0 * P: (q0 + qn) * P].rearrange(
                            "p (g e) -> p g e", e=P
                        ),
                        o2_ag[cl0["r"] * cfg.rs: cl0["r"] * cfg.rs
                              + rrows(cl0["r"]), :],
                        l0_sb[:, (b00 + q0) * 8: (b00 + q0 + qn) * 8],
                        qn * P,
                        qn * P,
                        P,
                        queue_num=next_q(),
                    )
                gb = lp.tile([P, LB * P], BF, tag="gb")
                for cl in lcalls1:
                    if not (b00 <= cl["blk0"] < b00 + nb0):
                        continue
                    r, nb = cl["r"], cl["nb"]
                    o = cl["blk0"] - b00
                    for q0 in range(0, nb, GMAX):
                        qn = min(GMAX, nb - q0)
                        nc.gpsimd.dma_gather(
                            gb[:, (o + q0) * P: (o + q0 + qn) * P].rearrange(
                                "p (g e) -> p g e", e=P
                            ),
                            o2_ag[r * cfg.rs: r * cfg.rs + rrows(r), :],
                            l1_sb[:, (cl["blk0"] + q0) * 8:
                                  (cl["blk0"] + q0 + qn) * 8],
                            qn * P,
                            qn * P,
                            P,
                            queue_num=next_q(),
                        )
                return ga, gb

            if phase >= 5:
                for cl0 in lcalls0:
                    ga, gb = lab_gather_region(cl0)
                    for ci in range(cl0["nb"]):
                        c = cl0["blk0"] + ci
                        sl = slice(ci * P, (ci + 1) * P)
                        prod = lp.tile([P, P], BF, tag="prod")
                        nc.vector.tensor_tensor(
                            out=prod[:], in0=ga[:, sl], in1=gb[:, sl],
                            op=OP.mult,
                        )
                        scr = lp.tile([P, P], BF, tag="scr")
                        nc.vector.tensor_tensor(
                            out=scr[:], in0=prod[:], in1=wv_sb[:], op=OP.mult
                        )
                        nc.vector.reduce_sum(
                            res_sb[:, c: c + 1], scr[:],
                            axis=mybir.AxisListType.X,
                        )
                nc.vector.tensor_scalar_add(
                    res_sb[:], res_sb[:], float(linb_sum)
                )
                nc.sync.dma_start(out=res_d[:, :], in_=res_sb[:])

    nc.finalize()
    return nc


# ------------------------------------------------------------------ driver


def make_in_maps(cfg, prep, W1, b1, W2, b2, lin_W, lin_b):
    consts = dict(
        ident=np.eye(P, dtype=np.float32).astype(ml_dtypes.bfloat16),
        w1=W1.astype(np.float32).astype(ml_dtypes.bfloat16),
        w2=W2.astype(np.float32).astype(ml_dtypes.bfloat16),
        b1bc=np.tile(b1.astype(np.float32)[None, :], (P, 1)),
        b2bc=np.tile(b2.astype(np.float32)[None, :], (P, 1)),
        wvbc=np.tile(
            lin_W.astype(np.float32).sum(axis=1)[None, :], (P, 1)
        ).astype(ml_dtypes.bfloat16),
    )
    in_maps = []
    for q in range(NC):
        m = dict(consts)
        m.update(
            xT=np.ascontiguousarray(
                prep["xT"][:, q * cfg.n_loc: (q + 1) * cfg.n_loc]
            ),
            dstl=prep["dstl"][q],
            ewp=prep["ewp"][q],
            iota=prep["iota"],
            gidx=prep["gidx"][q],
            l0idx=prep["l0idx"][q],
            l1idx=prep["l1idx"][q],
        )
        in_maps.append(m)
    return in_maps


def assemble_output(cfg, prep, results):
    out = np.zeros(cfg.n_labels, np.float32)
    for q in range(NC):
        r = np.asarray(results[q]["res"], np.float32)  # [128, LCP]
        flat = r.T.reshape(-1)  # slot j = c*128+p -> [c, p] row-major
        ids = prep["ids_slot"][q]
        m = ids >= 0
        out[ids[m]] = flat[m]
    return out


def run(cfg, x, edge_index, edge_weight, edge_label_index,
        W1, b1, W2, b2, lin_W, lin_b, trace=False, phase=99):
    global LAST_EXEC_NS, LAST_RESULTS
    prep = preprocess(cfg, np.asarray(x), np.asarray(edge_index),
                      np.asarray(edge_weight), np.asarray(edge_label_index))
    linb_sum = float(np.asarray(lin_b, np.float64).sum())
    nc = build_program(cfg, prep, linb_sum, phase=phase)
    in_maps = make_in_maps(cfg, prep, W1, b1, W2, b2, lin_W, lin_b)
    res = run_bass_kernel_spmd(
        nc, in_maps, list(range(NC)), trace=trace
    )
    LAST_EXEC_NS = res.exec_time_ns
    LAST_RESULTS = res
    return assemble_output(cfg, prep, res.results)


def kernel(x, edge_index, edge_weight, edge_label_index,
           W1, b1, W2, b2, lin_W, lin_b):
    trace = bool(os.environ.get("KERNEL_TRACE"))
    return run(FULL, x, edge_index, edge_weight, edge_label_index,
               W1, b1, W2, b2, lin_W, lin_b, trace=trace)


# revision 28
# speedup vs baseline: 1.5699x; 1.0078x over previous
"""GCN link-predictor kernel for 8 Trainium2 NeuronCores (Bass/Tile).

Strategy (SPMD, dst-sharded, v2):
  - Host: append self loops, assign each edge to the core owning its dst,
    group per 128-node dst tile, sort each tile's edges by src range
    (32768 rows = int16 dma_gather window), pad each (tile, range) to
    whole 128-edge chunks.  Ship the one-hot scatter matrices W_ew
    (W[e, dstl] = ew) pre-built in bf16, plus int16 local gather indices
    (16-partition wrapped) -- so the device does no one-hot construction.
  - Device per layer:  gather table rows with dma_gather (one call per
    supertile x src-range; ~1us SWDGE overhead amortized over thousands
    of rows), then chunk matmuls vs W_ew accumulate dst-tile partials in
    PSUM.  Layer1 table = dinv (.) (x @ W1) built from the core's own
    shard (lhsT = host-pre-transposed xT) and AllGathered; layer2 table
    = dinv (.) out1 produced directly at the layer1 evict (aggregate-
    then-GEMM reordering: (A X) W = A (X W)), so no full-table GEMM pass
    and no global dinv exchange -- all dinv folds are own-shard.
  - Labels: pairs sorted by (range(el0), range(el1)) into 16 groups so
    both sides gather via dma_gather; score = sum(a*b*wv) + sum(lin_b)
    with wv = lin_W @ 1.  Host un-permutes the result.
"""

import os
import sys

import numpy as np

for _p in ("/opt/trn_rl_repo",):
    if _p not in sys.path:
        sys.path.insert(0, _p)

import ml_dtypes  # noqa: E402

import concourse.bacc as bacc  # noqa: E402
import concourse.bass as bass  # noqa: E402
import concourse.mybir as mybir  # noqa: E402
from concourse.bass_utils import run_bass_kernel_spmd  # noqa: E402
from concourse.tile import TileContext  # noqa: E402

P = 128
NC = 8
RS = 32768  # dma_gather int16 window (rows)
SUP = 4    # dst tiles per supertile (gather-call granularity)
GMAX = 8   # max 128-row blocks per dma_gather call (SWDGE ring limit)
BF = mybir.dt.bfloat16
F32 = mybir.dt.float32
I16 = mybir.dt.int16

LAST_EXEC_NS = None
LAST_RESULTS = None


class Cfg:
    def __init__(self, n_nodes, n_labels, rs=RS):
        assert n_nodes % NC == 0
        self.n_nodes = n_nodes
        self.nodes_per_core = n_nodes // NC
        self.tiles_per_core = -(-self.nodes_per_core // P)
        self.n_loc = self.tiles_per_core * P
        self.n_pad = NC * self.n_loc
        self.n_labels = n_labels
        self.lab_per_core = -(-n_labels // NC)
        self.rs = rs
        self.nrg = -(-self.n_pad // rs)


FULL = Cfg(100000, 200000)


# ---------------------------------------------------------------- host prep


def _pad_ids(cfg, ids):
    q = np.minimum(ids // cfg.nodes_per_core, NC - 1)
    l = ids - q * cfg.nodes_per_core
    return q * cfg.n_loc + l


def _wrap16(flat_idx):
    # [n] -> [128, n//16]: idx j at [j%16, j//16], replicated to 128 parts
    n = len(flat_idx)
    assert n % 16 == 0
    a = np.zeros((16, n // 16), np.int16)
    a[np.arange(n) % 16, np.arange(n) // 16] = flat_idx
    return np.tile(a, (8, 1))


def preprocess(cfg, x, edge_index, edge_weight, edge_label_index):
    n = cfg.n_nodes
    T, NRG = cfg.tiles_per_core, cfg.nrg
    src = np.concatenate([edge_index[0], np.arange(n)]).astype(np.int64)
    dst = np.concatenate([edge_index[1], np.arange(n)]).astype(np.int64)
    ew = np.concatenate(
        [edge_weight.astype(np.float32), np.ones(n, np.float32)]
    )

    src_pad = _pad_ids(cfg, src)
    dst_pad = _pad_ids(cfg, dst)
    dq, dl = np.divmod(dst_pad, cfg.n_loc)
    dt_ = dl // P          # dst tile within core
    dloc = dl % P          # dst row within tile
    srange = src_pad // cfg.rs

    # chunk counts per (tile, range): max over cores for SPMD uniformity
    key = (dq * T + dt_) * NRG + srange
    counts = np.bincount(key, minlength=NC * T * NRG).reshape(NC, T, NRG)
    Kr = -(-counts.max(axis=0) // P)  # [T, NRG] blocks (may be 0)

    # supertile slab layout: for each supertile, range-major block order
    sups = []
    boff = np.zeros((T, NRG), np.int64)  # global block offset of (t, r)
    gcol = 0  # running gidx column offset
    nblk = 0
    for t0 in range(0, T, SUP):
        tiles = list(range(t0, min(t0 + SUP, T)))
        calls = []
        slab0 = nblk
        tile_chunks = {t: [] for t in tiles}
        for r in range(NRG):
            nb = int(sum(Kr[t, r] for t in tiles))
            if nb == 0:
                continue
            call_start = nblk
            for t in tiles:
                boff[t, r] = nblk
                tile_chunks[t].extend(range(nblk, nblk + int(Kr[t, r])))
                nblk += int(Kr[t, r])
            calls.append(dict(r=r, nb=nb, blk0=call_start, gcol=gcol))
            gcol += nb * 8  # nb*128/16 cols
        sups.append(dict(tiles=tiles, calls=calls, slab0=slab0,
                         nblk=nblk - slab0,
                         tile_chunks={t: [b - slab0 for b in tile_chunks[t]]
                                      for t in tiles}))
    C = nblk

    # slot assignment for every edge
    order = np.argsort(key, kind="stable")
    sk = key[order]
    starts = np.zeros(NC * T * NRG + 1, np.int64)
    starts[1:] = np.cumsum(counts.reshape(-1))
    pos = np.arange(len(order)) - starts[sk]
    e_core = order * 0 + dq[order]
    e_t, e_r = dt_[order], srange[order]
    blk = boff[e_t, e_r] + pos // P
    part = pos % P

    # per-slot dst-local index and edge weight, [P, C] each (slab order)
    slot_lin = blk * P + part
    dstl_a = np.zeros((NC, C * P), ml_dtypes.bfloat16)
    ewp_a = np.zeros((NC, C * P), ml_dtypes.bfloat16)
    dstl_a[e_core, slot_lin] = dloc[order].astype(ml_dtypes.bfloat16)
    ewp_a[e_core, slot_lin] = ew[order].astype(ml_dtypes.bfloat16)
    dstl_a = np.ascontiguousarray(
        dstl_a.reshape(NC, C, P).transpose(0, 2, 1))
    ewp_a = np.ascontiguousarray(
        ewp_a.reshape(NC, C, P).transpose(0, 2, 1))

    # gidx: per-call wrapped int16 local indices (idx j of a call lands at
    # [j%16, call_gcol + j//16], replicated 8x across partition groups)
    cid_blk0 = np.zeros((T, NRG), np.int64)
    cid_gcol = np.zeros((T, NRG), np.int64)
    for s in sups:
        for cl in s["calls"]:
            for t in s["tiles"]:
                cid_blk0[t, cl["r"]] = cl["blk0"]
                cid_gcol[t, cl["r"]] = cl["gcol"]
    loc_idx = (src_pad[order] - e_r * cfg.rs).astype(np.int64)
    j = (blk - cid_blk0[e_t, e_r]) * P + part
    gidx16 = np.zeros((NC, 16, gcol), np.int16)
    gidx16[e_core, j % 16, cid_gcol[e_t, e_r] + j // 16] = loc_idx
    gidx = np.tile(gidx16, (1, 8, 1))

    # ---- labels: sort by (range(el0), range(el1)) into NRG^2 groups ----
    el = edge_label_index.astype(np.int64)
    el0 = _pad_ids(cfg, el[0])
    el1 = _pad_ids(cfg, el[1])
    lpc = cfg.lab_per_core
    NG = NRG * NRG
    lab_ids = []      # per core: original label index per slot (-1 pad)
    lcounts = np.zeros((NC, NG), np.int64)
    per_core = []
    for q in range(NC):
        lo, hi = q * lpc, min((q + 1) * lpc, cfg.n_labels)
        ids = np.arange(lo, hi)
        g = (el0[ids] // cfg.rs) * NRG + (el1[ids] // cfg.rs)
        o = np.argsort(g, kind="stable")
        per_core.append((ids[o], g[o]))
        lcounts[q] = np.bincount(g, minlength=NG)
    Lg = -(-lcounts.max(axis=0) // P)  # [NG] blocks, max over cores
    LCP = int(Lg.sum())
    g_blk0 = np.zeros(NG + 1, np.int64)
    g_blk0[1:] = np.cumsum(Lg)

    l0flat = np.zeros((NC, LCP * P), np.int64)
    l1flat = np.zeros((NC, LCP * P), np.int64)
    ids_slot = -np.ones((NC, LCP * P), np.int64)
    for q in range(NC):
        ids_o, g_o = per_core[q]
        gstart = np.zeros(NG + 1, np.int64)
        gstart[1:] = np.cumsum(lcounts[q])
        posl = np.arange(len(ids_o)) - gstart[g_o]
        slot = g_blk0[g_o] * P + posl
        l0flat[q, slot] = el0[ids_o] - (g_o // NRG) * cfg.rs
        l1flat[q, slot] = el1[ids_o] - (g_o % NRG) * cfg.rs
        ids_slot[q, slot] = ids_o
    l0idx = np.stack([_wrap16(l0flat[q]) for q in range(NC)])
    l1idx = np.stack([_wrap16(l1flat[q]) for q in range(NC)])

    # label gather calls: el0 side = NRG calls (groups r0*NRG..r0*NRG+NRG-1),
    # el1 side = NG calls
    lcalls0 = []
    for r0 in range(NRG):
        nb = int(Lg[r0 * NRG: (r0 + 1) * NRG].sum())
        if nb:
            lcalls0.append(dict(r=r0, nb=nb, blk0=int(g_blk0[r0 * NRG])))
    lcalls1 = []
    for g in range(NG):
        nb = int(Lg[g])
        if nb:
            lcalls1.append(dict(r=g % NRG, nb=nb, blk0=int(g_blk0[g])))

    # node features: padded, transposed, own-shard sliced per core
    x_pad = np.zeros((cfg.n_pad, P), np.float32)
    x_pad[_pad_ids(cfg, np.arange(n))] = x
    xT = np.ascontiguousarray(x_pad.T).astype(ml_dtypes.bfloat16)

    bmax = max(su["nblk"] for su in sups)
    iota_rep = np.tile(np.arange(P, dtype=np.float32), (P, bmax)).astype(
        ml_dtypes.bfloat16)
    return dict(dstl=dstl_a, ewp=ewp_a, iota=iota_rep, gidx=gidx,
                l0idx=l0idx, l1idx=l1idx, xT=xT,
                sups=sups, C=C, LCP=LCP, lcalls0=lcalls0, lcalls1=lcalls1,
                ids_slot=ids_slot)


# ------------------------------------------------------------- bass program


def build_program(cfg, prep, linb_sum, phase=99):
    T, NRG = cfg.tiles_per_core, cfg.nrg
    NPAD, NLOC = cfg.n_pad, cfg.n_loc
    sups, C, LCP = prep["sups"], prep["C"], prep["LCP"]
    lcalls0, lcalls1 = prep["lcalls0"], prep["lcalls1"]
    GCOL = prep["gidx"].shape[2]
    BMAX = max(s["nblk"] for s in sups)
    rg = [list(range(NC))]

    def rrows(r):
        return min(cfg.rs, NPAD - r * cfg.rs)

    nc = bacc.Bacc(None, target_bir_lowering=False, debug=False,
                   num_swdge_queues=4)

    xT_d = nc.declare_dram_parameter("xT", [P, NLOC], BF, False)
    dstl_d = nc.declare_dram_parameter("dstl", [P, C], BF, False)
    ewp_d = nc.declare_dram_parameter("ewp", [P, C], BF, False)
    iota_d = nc.declare_dram_parameter("iota", [P, BMAX * P], BF, False)
    gidx_d = nc.declare_dram_parameter("gidx", [P, GCOL], I16, False)
    l0_d = nc.declare_dram_parameter("l0idx", [P, LCP * 8], I16, False)
    l1_d = nc.declare_dram_parameter("l1idx", [P, LCP * 8], I16, False)
    ident_d = nc.declare_dram_parameter("ident", [P, P], BF, False)
    w1_d = nc.declare_dram_parameter("w1", [P, P], BF, False)
    w2_d = nc.declare_dram_parameter("w2", [P, P], BF, False)
    b1_d = nc.declare_dram_parameter("b1bc", [P, P], F32, False)
    b2_d = nc.declare_dram_parameter("b2bc", [P, P], F32, False)
    wv_d = nc.declare_dram_parameter("wvbc", [P, P], BF, False)
    res_d = nc.declare_dram_parameter("res", [P, LCP], F32, True)

    t1_sh = nc.dram_tensor("t1_sh", [NLOC, P], BF)
    t1_ag = nc.dram_tensor("t1_ag", [NPAD, P], BF)
    t2_sh = nc.dram_tensor("t2_sh", [NLOC, P], BF)
    t2_ag = nc.dram_tensor("t2_ag", [NPAD, P], BF)
    o2_sh = nc.dram_tensor("o2_sh", [NLOC, P], BF)
    o2_ag = nc.dram_tensor("o2_ag", [NPAD, P], BF)

    AF = mybir.ActivationFunctionType
    OP = mybir.AluOpType
    qctr = [0]

    def next_q():
        qctr[0] = (qctr[0] + 1) % 4
        return qctr[0]

    with TileContext(nc) as tc:
        with (
            tc.tile_pool(name="const", bufs=1) as cp,
            tc.tile_pool(name="wslab", bufs=2) as wp,
            tc.tile_pool(name="gbuf", bufs=2) as gp,
            tc.tile_pool(name="idx", bufs=8) as ip,
            tc.tile_pool(name="xtile", bufs=3) as xp,
            tc.tile_pool(name="evict", bufs=4) as ep,
            tc.tile_pool(name="lab", bufs=1) as lp,
            tc.tile_pool(name="ps_deg", bufs=1, space="PSUM") as psd,
            tc.tile_pool(name="ps_agg", bufs=2, space="PSUM") as psa,
            tc.tile_pool(name="ps_gem", bufs=2, space="PSUM") as psg,
            tc.tile_pool(name="ps_tr", bufs=2, space="PSUM") as pst,
        ):
            # ---- persistent SBUF ----
            ident_sb = cp.tile([P, P], BF)
            nc.sync.dma_start(out=ident_sb[:], in_=ident_d[:, :])
            w1_sb = cp.tile([P, P], BF)
            nc.sync.dma_start(out=w1_sb[:], in_=w1_d[:, :])
            w2_sb = cp.tile([P, P], BF)
            nc.sync.dma_start(out=w2_sb[:], in_=w2_d[:, :])
            b1_sb = cp.tile([P, P], F32)
            nc.sync.dma_start(out=b1_sb[:], in_=b1_d[:, :])
            b2_sb = cp.tile([P, P], F32)
            nc.sync.dma_start(out=b2_sb[:], in_=b2_d[:, :])
            wv_sb = cp.tile([P, P], BF)
            nc.sync.dma_start(out=wv_sb[:], in_=wv_d[:, :])
            l0_sb = cp.tile([P, LCP * 8], I16)
            nc.sync.dma_start(out=l0_sb[:], in_=l0_d[:, :])
            l1_sb = cp.tile([P, LCP * 8], I16)
            nc.sync.dma_start(out=l1_sb[:], in_=l1_d[:, :])
            dstl_sb = cp.tile([P, C], BF)
            nc.sync.dma_start(out=dstl_sb[:], in_=dstl_d[:, :])
            ewp_sb = cp.tile([P, C], BF)
            nc.sync.dma_start(out=ewp_sb[:], in_=ewp_d[:, :])
            iota_sb = cp.tile([P, BMAX * P], BF)
            nc.sync.dma_start(out=iota_sb[:], in_=iota_d[:, :])
            iota3 = iota_sb[:].rearrange("p (g e) -> p g e", e=P)
            deg_sb = cp.tile([P, T], F32)
            rec_sb = cp.tile([P, T], F32)
            dinv_own = cp.tile([P, T], F32)
            res_sb = cp.tile([P, LCP], F32)

            def build_wslab(s, fold_ew):
                # one-hot W for the whole supertile slab in 1-2 DVE ops:
                # W[p, b, j] = (j == dstl[p, b]) [* ew[p, b]]
                w = wp.tile([P, BMAX * P], BF, tag="w")
                nb = s["nblk"]
                c0 = s["slab0"]
                w3 = w[:, : nb * P].rearrange("p (g e) -> p g e", e=P)
                nc.vector.tensor_tensor(
                    out=w3,
                    in0=iota3[:, :nb, :],
                    in1=dstl_sb[:, c0: c0 + nb].to_broadcast([P, nb, P]),
                    op=OP.is_equal,
                )
                if fold_ew:
                    nc.vector.tensor_tensor(
                        out=w3,
                        in0=w3,
                        in1=ewp_sb[:, c0: c0 + nb].to_broadcast([P, nb, P]),
                        op=OP.mult,
                    )
                return w

            # ---- deg pass (own tiles) ----
            for s in sups:
                w = build_wslab(s, False)
                for t in s["tiles"]:
                    blks = s["tile_chunks"][t]
                    pd = psd.tile([P, 1], F32)
                    for i, b in enumerate(blks):
                        c = s["slab0"] + b
                        nc.tensor.matmul(
                            out=pd[:],
                            lhsT=w[:, b * P: (b + 1) * P],
                            rhs=ewp_sb[:, c: c + 1],
                            start=(i == 0),
                            stop=(i == len(blks) - 1),
                        )
                    nc.scalar.activation(deg_sb[:, t: t + 1], pd[:], AF.Copy)
            nc.vector.tensor_scalar_max(deg_sb[:], deg_sb[:], 1.0)
            nc.vector.reciprocal(rec_sb[:], deg_sb[:])
            nc.scalar.activation(dinv_own[:], rec_sb[:], AF.Sqrt)

            # ---- layer-1 table: own shard of dinv*(x@W1), then AllGather
            for t in range(T):
                lhsT = xp.tile([P, P], BF, tag="lhsT")
                nc.sync.dma_start(
                    out=lhsT[:], in_=xT_d[:, t * P: (t + 1) * P]
                )
                pg = psg.tile([P, P], F32, tag="pg")
                nc.tensor.matmul(
                    out=pg[:], lhsT=lhsT[:], rhs=w1_sb[:],
                    start=True, stop=True,
                )
                hbf = xp.tile([P, P], BF, tag="hbf")
                nc.scalar.activation(
                    hbf[:], pg[:], AF.Copy, scale=dinv_own[:, t: t + 1]
                )
                nc.sync.dma_start(
                    out=t1_sh[t * P: (t + 1) * P, :], in_=hbf[:]
                )
            nc.gpsimd.collective_compute(
                "AllGather", OP.bypass, replica_groups=rg,
                ins=[t1_sh[:, :]], outs=[t1_ag[:, :]],
            )
            if phase == 1:
                pr = cp.tile([P, LCP], F32)
                nc.vector.tensor_copy(pr[:], dinv_own[:, :1].to_broadcast([P, LCP]))
                nc.sync.dma_start(out=res_d[:, :], in_=pr[:])

            # ---- aggregation supertile machinery ----
            def gather_sup(s, table):
                g = gp.tile([P, BMAX * P], BF, tag="g")
                for cl in s["calls"]:
                    r, nb = cl["r"], cl["nb"]
                    b0 = cl["blk0"] - s["slab0"]
                    it = ip.tile([P, BMAX * 8], I16, tag="gi")
                    nc.sync.dma_start(
                        out=it[:, : nb * 8],
                        in_=gidx_d[:, cl["gcol"]: cl["gcol"] + nb * 8],
                    )
                    for q0 in range(0, nb, GMAX):
                        qn = min(GMAX, nb - q0)
                        nc.gpsimd.dma_gather(
                            g[:, (b0 + q0) * P: (b0 + q0 + qn) * P].rearrange(
                                "p (g e) -> p g e", e=P
                            ),
                            table[r * cfg.rs: r * cfg.rs + rrows(r), :],
                            it[:, q0 * 8: (q0 + qn) * 8],
                            qn * P,
                            qn * P,
                            P,
                            queue_num=next_q(),
                        )
                return g

            # ---- layer 1: aggregate t1 -> out1, emit t2 = dinv*out1 ----
            def agg1():
                for s in sups:
                    w = build_wslab(s, True)
                    g = gather_sup(s, t1_ag)
                    for t in s["tiles"]:
                        blks = s["tile_chunks"][t]
                        pa = psa.tile([P, P], F32)
                        for i, b in enumerate(blks):
                            sl = slice(b * P, (b + 1) * P)
                            nc.tensor.matmul(
                                out=pa[:], lhsT=w[:, sl], rhs=g[:, sl],
                                start=(i == 0), stop=(i == len(blks) - 1),
                            )
                        t1 = ep.tile([P, P], F32, tag="t1")
                        nc.scalar.activation(
                            t1[:], pa[:], AF.Copy,
                            scale=dinv_own[:, t: t + 1],
                        )
                        nc.vector.tensor_tensor(
                            out=t1[:], in0=t1[:], in1=b1_sb[:], op=OP.add
                        )
                        o1 = ep.tile([P, P], F32, tag="o1")
                        nc.scalar.activation(o1[:], t1[:], AF.Relu)
                        t2b = ep.tile([P, P], BF, tag="t2b")
                        nc.scalar.activation(
                            t2b[:], o1[:], AF.Copy,
                            scale=dinv_own[:, t: t + 1],
                        )
                        nc.sync.dma_start(
                            out=t2_sh[t * P: (t + 1) * P, :], in_=t2b[:]
                        )

            # ---- layer 2: aggregate t2 (f-major psum), GEMM W2, evict ----
            def agg2():
                for s in sups:
                    w = build_wslab(s, True)
                    g = gather_sup(s, t2_ag)
                    for t in s["tiles"]:
                        blks = s["tile_chunks"][t]
                        pa = psa.tile([P, P], F32)
                        for i, b in enumerate(blks):
                            sl = slice(b * P, (b + 1) * P)
                            nc.tensor.matmul(
                                out=pa[:], lhsT=g[:, sl], rhs=w[:, sl],
                                start=(i == 0), stop=(i == len(blks) - 1),
                            )
                        uT = ep.tile([P, P], BF, tag="uT")
                        nc.scalar.activation(uT[:], pa[:], AF.Copy)
                        pb = psg.tile([P, P], F32, tag="pg")
                        nc.tensor.matmul(
                            out=pb[:], lhsT=w2_sb[:], rhs=uT[:],
                            start=True, stop=True,
                        )
                        vT = ep.tile([P, P], BF, tag="vT")
                        nc.scalar.activation(vT[:], pb[:], AF.Copy)
                        pt = pst.tile([P, P], BF)
                        nc.tensor.transpose(
                            out=pt[:], in_=vT[:], identity=ident_sb[:]
                        )
                        t1 = ep.tile([P, P], F32, tag="t1")
                        nc.scalar.activation(
                            t1[:], pt[:], AF.Copy,
                            scale=dinv_own[:, t: t + 1],
                        )
                        nc.vector.tensor_tensor(
                            out=t1[:], in0=t1[:], in1=b2_sb[:], op=OP.add
                        )
                        o2 = ep.tile([P, P], BF, tag="o2")
                        nc.scalar.activation(o2[:], t1[:], AF.Relu)
                        nc.sync.dma_start(
                            out=o2_sh[t * P: (t + 1) * P, :], in_=o2[:]
                        )

            def probe(src):
                pr = cp.tile([P, P], BF)
                nc.sync.dma_start(out=pr[:], in_=src)
                prf = cp.tile([P, P], F32)
                nc.vector.tensor_copy(prf[:], pr[:])
                nc.sync.dma_start(out=res_d[:, : min(LCP, P)],
                                  in_=prf[:, : min(LCP, P)])

            if phase >= 2:
                agg1()
            if phase == 2:
                probe(t2_sh[0:P, :])
            if phase >= 3:
                nc.gpsimd.collective_compute(
                    "AllGather", OP.bypass, replica_groups=rg,
                    ins=[t2_sh[:, :]], outs=[t2_ag[:, :]],
                )
            if phase == 3:
                probe(t2_ag[0:P, :])
            if phase >= 4:
                agg2()
                nc.gpsimd.collective_compute(
                    "AllGather", OP.bypass, replica_groups=rg,
                    ins=[o2_sh[:, :]], outs=[o2_ag[:, :]],
                )
            if phase == 4:
                probe(o2_ag[0:P, :])

            # ---- label pass: process one el0-range region at a time ----
            LB = max((cl["nb"] for cl in lcalls0), default=1)

            def lab_gather_region(cl0):
                b00, nb0 = cl0["blk0"], cl0["nb"]
                ga = lp.tile([P, LB * P], BF, tag="ga")
                for q0 in range(0, nb0, GMAX):
                    qn = min(GMAX, nb0 - q0)
                    nc.gpsimd.dma_gather(
                        ga[:, q0 * P: (q0 + qn) * P].rearrange(
                            "p (g e) -> p g e", e=P
                        ),
                        o2_ag[cl0["r"] * cfg.rs: cl0["r"] * cfg.rs
                              + rrows(cl0["r"]), :],
                        l0_sb[:, (b00 + q0) * 8: (b00 + q0 + qn) * 8],
                        qn * P,
                        qn * P,
                        P,
                        queue_num=next_q(),
                    )
                gb = lp.tile([P, LB * P], BF, tag="gb")
                for cl in lcalls1:
                    if not (b00 <= cl["blk0"] < b00 + nb0):
                        continue
                    r, nb = cl["r"], cl["nb"]
                    o = cl["blk0"] - b00
                    for q0 in range(0, nb, GMAX):
                        qn = min(GMAX, nb - q0)
                        nc.gpsimd.dma_gather(
                            gb[:, (o + q0) * P: (o + q0 + qn) * P].rearrange(
                                "p (g e) -> p g e", e=P
                            ),
                            o2_ag[r * cfg.rs: r * cfg.rs + rrows(r), :],
                            l1_sb[:, (cl["blk0"] + q0) * 8:
                                  (cl["blk0"] + q0 + qn) * 8],
                            qn * P,
                            qn * P,
                            P,
                            queue_num=next_q(),
                        )
                return ga, gb

            if phase >= 5:
                for cl0 in lcalls0:
                    ga, gb = lab_gather_region(cl0)
                    for ci in range(cl0["nb"]):
                        c = cl0["blk0"] + ci
                        sl = slice(ci * P, (ci + 1) * P)
                        prod = lp.tile([P, P], BF, tag="prod")
                        nc.vector.tensor_tensor(
                            out=prod[:], in0=ga[:, sl], in1=gb[:, sl],
                            op=OP.mult,
                        )
                        scr = lp.tile([P, P], BF, tag="scr")
                        nc.vector.tensor_tensor(
                            out=scr[:], in0=prod[:], in1=wv_sb[:], op=OP.mult
                        )
                        nc.vector.reduce_sum(
                            res_sb[:, c: c + 1], scr[:],
                            axis=mybir.AxisListType.X,
                        )
                nc.vector.tensor_scalar_add(
                    res_sb[:], res_sb[:], float(linb_sum)
                )
                nc.sync.dma_start(out=res_d[:, :], in_=res_sb[:])

    nc.finalize()
    return nc


# ------------------------------------------------------------------ driver


def make_in_maps(cfg, prep, W1, b1, W2, b2, lin_W, lin_b):
    consts = dict(
        ident=np.eye(P, dtype=np.float32).astype(ml_dtypes.bfloat16),
        w1=W1.astype(np.float32).astype(ml_dtypes.bfloat16),
        w2=W2.astype(np.float32).astype(ml_dtypes.bfloat16),
        b1bc=np.tile(b1.astype(np.float32)[None, :], (P, 1)),
        b2bc=np.tile(b2.astype(np.float32)[None, :], (P, 1)),
        wvbc=np.tile(
            lin_W.astype(np.float32).sum(axis=1)[None, :], (P, 1)
        ).astype(ml_dtypes.bfloat16),
    )
    in_maps = []
    for q in range(NC):
        m = dict(consts)
        m.update(
            xT=np.ascontiguousarray(
                prep["xT"][:, q * cfg.n_loc: (q + 1) * cfg.n_loc]
            ),
            dstl=prep["dstl"][q],
            ewp=prep["ewp"][q],
            iota=prep["iota"],
            gidx=prep["gidx"][q],
            l0idx=prep["l0idx"][q],
            l1idx=prep["l1idx"][q],
        )
        in_maps.append(m)
    return in_maps


def assemble_output(cfg, prep, results):
    out = np.zeros(cfg.n_labels, np.float32)
    for q in range(NC):
        r = np.asarray(results[q]["res"], np.float32)  # [128, LCP]
        flat = r.T.reshape(-1)  # slot j = c*128+p -> [c, p] row-major
        ids = prep["ids_slot"][q]
        m = ids >= 0
        out[ids[m]] = flat[m]
    return out


def run(cfg, x, edge_index, edge_weight, edge_label_index,
        W1, b1, W2, b2, lin_W, lin_b, trace=False, phase=99):
    global LAST_EXEC_NS, LAST_RESULTS
    prep = preprocess(cfg, np.asarray(x), np.asarray(edge_index),
                      np.asarray(edge_weight), np.asarray(edge_label_index))
    linb_sum = float(np.asarray(lin_b, np.float64).sum())
    nc = build_program(cfg, prep, linb_sum, phase=phase)
    in_maps = make_in_maps(cfg, prep, W1, b1, W2, b2, lin_W, lin_b)
    res = run_bass_kernel_spmd(
        nc, in_maps, list(range(NC)), trace=trace
    )
    LAST_EXEC_NS = res.exec_time_ns
    LAST_RESULTS = res
    return assemble_output(cfg, prep, res.results)


def kernel(x, edge_index, edge_weight, edge_label_index,
           W1, b1, W2, b2, lin_W, lin_b):
    trace = bool(os.environ.get("KERNEL_TRACE"))
    return run(FULL, x, edge_index, edge_weight, edge_label_index,
               W1, b1, W2, b2, lin_W, lin_b, trace=trace)
